# revision 31
# baseline (speedup 1.0000x reference)
"""Trainium2 Bass kernel for single-head cross-attention (v3: folded weights).

Reference computation (B=4, Sq=Skv=2048, D=1024, fp32):
    Q = query @ Wq + bq ; K = key @ Wk + bk ; V = value @ Wv + bv
    out = softmax(Q K^T / sqrt(D)) V @ Wo + bo

Since no nonlinearity separates the projections from the score/output
matmuls, the host folds the weights (a static, per-model transform):
    M = Wq @ Wk.T        scores = (query @ M) @ key^T  (K proj eliminated)
    N = Wv @ Wo          out    = (attn @ value) @ N   (V proj eliminated)
Bias terms fold exactly: the per-kv offset key @ (Wk @ bq) becomes the
exp() activation bias; per-q offsets cancel against the softmax
denominator (we divide by the sums at the very end, so they never need
computing); bv @ Wo + bo is the output bias.

This removes 25% of the device FLOPs and, because each core can simply
be HANDED the full raw key/value for its batch, the K/V AllGathers of
v2 disappear entirely. Sharding: 8 shards = (batch b) x (query half h);
core 2*b+h computes output rows [h*1024,(h+1)*1024) of batch b. All
matmul operands are bf16 (fp8 DoubleRow was measured at 1.9e-2 rel err
against the 2e-2 budget - too close).

M is pre-scaled by 32 on the host so Q' = query @ 32M has entries O(15)
(fp32 PSUM doesn't care, but it keeps the bf16 store well-conditioned);
the exp scale absorbs the 2^-10.

Dataflow per core (all contractions land on SBUF partitions):
    Q'^T[e,q]  = M32.T @ qT        (lhsT=m32,  rhs=qT)
    S^T[kv,q]  = key @ Q'^T        (lhsT=kT,   rhs=Q'^T)
    A^T        = exp(S^T/1024 + t2s)            (unnormalized)
    AX^T[dv,q] = value.T @ A^T     (lhsT=xv,   rhs=A^T)
    sums[q,1]  = A @ ones          (lhsT=A^T,  rhs=ones)
    F[q,f]     = AX @ N            (lhsT=AX^T, rhs=n2)
    out        = F * (1/sums) + (bv @ Wo + bo)
"""

import sys

if "/opt/trn_rl_repo" not in sys.path:
    sys.path.insert(0, "/opt/trn_rl_repo")

from contextlib import ExitStack

import ml_dtypes
import numpy as np

import concourse.bass as bass
import concourse.mybir as mybir
import concourse.tile as tile
from concourse import bacc
from concourse.bass_utils import run_bass_kernel_spmd

B, SQ, SKV, D = 4, 2048, 2048, 1024
NCORES = 8
QL = SQ // 2  # local query rows per core
P = 128
DC = D // P  # feature chunks (8)
KVC = SKV // P  # kv chunks (16)
N5 = 512
NQB = QL // N5  # query blocks (2)
F32 = mybir.dt.float32
CDT = mybir.dt.bfloat16
F8 = mybir.dt.float8e4
NP_CDT = ml_dtypes.bfloat16
NP_F8 = ml_dtypes.float8_e4m3fn
MS = 32.0  # host pre-scale on M
SCALE = 1.0 / (32.0 * MS)  # exp scale: 1/sqrt(D) / MS

# kv chunks [KV8C:KVC) compute their scores in fp8e4 DoubleRow (2 k-rows per
# instruction, ~1.8x bf16 rate); softmax attenuates the quantization noise.
# Measured rel err 1.39e-2 at KV8C=8 vs the 2e-2 budget (all-bf16: 3.6e-3).
KV8C = 8  # first fp8 kv chunk; KVC to disable fp8 entirely
KVB = KV8C * P  # kv rows computed in bf16

AF = mybir.ActivationFunctionType
ALU = mybir.AluOpType
DR = mybir.MatmulPerfMode.DoubleRow


def _build_tile(ctx: ExitStack, tc, aps):
    nc = tc.nc
    qT, kTb, kT8, xv, m32, n2, t2s, bo2, out = aps

    big = ctx.enter_context(tc.tile_pool(name="big", bufs=1))
    attn_pool = ctx.enter_context(tc.tile_pool(name="attn", bufs=2))
    evac = ctx.enter_context(tc.tile_pool(name="evac", bufs=4))
    # Q' gets all 8 PSUM banks (one per ec) so each query block is a single
    # dc-outer sweep: the m32 walk then spans 13.9us of matmuls and stays
    # behind the DMA stream. The pool closes before psum/psum_s open.
    qpool_cm = tc.tile_pool(name="qpool", bufs=8, space="PSUM")
    qpool = qpool_cm.__enter__()

    # ---- input DMAs, spread across rings so they stream in parallel --------
    # Critical path: the first Q' psum group consumes every d-chunk of m32,
    # so m32 rides two rings (evens/odds) and qT's first query block leads
    # the third; kT (scores, needed ~15us in) gets the sync ring to itself.
    m32_r = m32.rearrange("(c p) e -> p c e", p=P)
    qT_r = qT.rearrange("(c p) q -> p c q", p=P)
    kTb_r = kTb.rearrange("(c p) n -> p c n", p=P)
    kT8_r = kT8.rearrange("(c p) n -> p c n", p=P)
    xv_r = xv.rearrange("(c p) n -> p c n", p=P)
    n2_r = n2.rearrange("(c p) f -> p c f", p=P)

    # Per-d-chunk DMAs: HWDGE descriptor-generation time is linear in the
    # number of contiguous runs, so one [:, :, slice] DMA costs the same
    # sequencer time as eight [:, c, :] DMAs but delays every consumer until
    # the whole thing is issued. The SDMA engines round-robin between rings
    # that have queued work at packet granularity, so EVERY ring must carry
    # earliest-deadline traffic first: stripe all tensors across the three
    # rings in global consumption order (m32/qT -> kT -> xv -> n2).
    m32_s = big.tile([P, DC, D], CDT, tag="m32")
    qT_s = big.tile([P, DC, QL], CDT, tag="qT")
    kTb_s = big.tile([P, DC, KVB], CDT, tag="kTb")
    kT8_s = big.tile([P, DC, SKV - KVB], F8, tag="kT8")
    xv_s = big.tile([P, KVC, D], CDT, tag="xv")
    n2_s = big.tile([P, DC, D], CDT, tag="n2")
    t2s_s = big.tile([P, KVC], F32, tag="t2s")
    bo2_s = big.tile([P, D], F32, tag="bo2")
    bo2_bcast = bass.AP(tensor=bo2.tensor, offset=bo2.offset, ap=[[0, P], bo2.ap[0]])
    ones = big.tile([P, 1], CDT, tag="ones")
    nc.vector.memset(ones, 1.0)

    # Phases 1-2 (before the exp stream occupies Scalar) use all three rings;
    # later phases avoid the scalar ring so DMA issues never delay the
    # exp/evac activations queued behind them.
    xfers = []
    for dc in range(DC):  # phase 1: Q' operands, in dc consumption order
        xfers.append((m32_s[:, dc, :], m32_r[:, dc, :]))
        xfers.append((qT_s[:, dc, 0:N5], qT_r[:, dc, 0:N5]))
    xfers.append((t2s_s, t2s))  # host ships t2s pre-transposed to [P, KVC]
    for dc in range(DC):  # phase 2a: qb1 queries (consumed from ~+23us)
        xfers.append((qT_s[:, dc, N5:QL], qT_r[:, dc, N5:QL]))
    for dc in range(DC):  # phase 2b: scores operands (consumed from ~+38us)
        xfers.append((kTb_s[:, dc, :], kTb_r[:, dc, :]))
        xfers.append((kT8_s[:, dc, :], kT8_r[:, dc, :]))
    rings = [nc.sync, nc.scalar, nc.gpsimd]
    for i, (dst, src) in enumerate(xfers):
        rings[i % 3].dma_start(out=dst, in_=src)

    xfers2 = []
    for c in range(KVC):  # phase 3: AX operand
        xfers2.append((xv_s[:, c, :], xv_r[:, c, :]))
    xfers2.append((bo2_s, bo2_bcast))
    for dc in range(DC):  # phase 4: output projection operand
        xfers2.append((n2_s[:, dc, :], n2_r[:, dc, :]))
    rings2 = [nc.sync, nc.gpsimd]
    for i, (dst, src) in enumerate(xfers2):
        rings2[i % 2].dma_start(out=dst, in_=src)

    # ---- Q'^T = M32.T @ qT ---------------------------------------------------
    qp = big.tile([P, DC, QL], CDT, tag="qp")  # Q'^T: [d'%128, d'//128, q]
    qp8 = None
    if KV8C < KVC:
        qp8 = big.tile([P, DC, QL], F8, tag="qp8", name="qp8")

    def qp_evac(ec, qb, ps):
        # alternate engines so the 8-deep evac drain halves in latency; the
        # fp8 copy of Q' feeds the DoubleRow half of the scores matmul.
        sl = slice(qb * N5, (qb + 1) * N5)
        dsts = [qp] if qp8 is None else ([qp, qp8] if ec % 2 == 0 else [qp8, qp])
        for i, dst in enumerate(dsts):
            if i == 0:
                nc.scalar.activation(
                    out=dst[:, ec, sl], in_=ps, func=AF.Identity, scale=1.0
                )
            else:
                nc.vector.tensor_copy(out=dst[:, ec, sl], in_=ps)

    def qprime8(qb):
        pss = [
            qpool.tile([P, N5], F32, tag="qmm", name=f"qps{qb}_{ec}")
            for ec in range(DC)
        ]
        for dc in range(DC):
            for ec in range(DC):
                nc.tensor.matmul(
                    pss[ec],
                    lhsT=m32_s[:, dc, ec * P : (ec + 1) * P],
                    rhs=qT_s[:, dc, qb * N5 : (qb + 1) * N5],
                    start=(dc == 0),
                    stop=(dc == DC - 1),
                )
        for ec in range(DC):
            qp_evac(ec, qb, pss[ec])

    # ---- scores + exp + sums + AX, one kv pass per 512-query block ----------
    def scores_exp(qb):
        attnT = attn_pool.tile([P, KVC, N5], CDT, tag="attnT")
        qsl = slice(qb * N5, (qb + 1) * N5)
        for c in range(KVC):
            ps = psum.tile([P, N5], F32, tag="mm")
            if c < KV8C:  # bf16 path
                for dc in range(DC):
                    nc.tensor.matmul(
                        ps,
                        lhsT=kTb_s[:, dc, c * P : (c + 1) * P],
                        rhs=qp[:, dc, qsl],
                        start=(dc == 0),
                        stop=(dc == DC - 1),
                    )
            else:  # fp8 DoubleRow path: two d-chunks per instruction
                c8 = c - KV8C
                for dc in range(0, DC, 2):
                    nc.tensor.matmul(
                        ps,
                        lhsT=kT8_s[:, dc : dc + 2, c8 * P : (c8 + 1) * P],
                        rhs=qp8[:, dc : dc + 2, qsl],
                        start=(dc == 0),
                        stop=(dc == DC - 2),
                        perf_mode=DR,
                    )
            nc.scalar.activation(
                out=attnT[:, c, :],
                in_=ps,
                func=AF.Exp,
                scale=SCALE,
                bias=t2s_s[:, c : c + 1],
            )
        return attnT

    def sums_recip(attnT):
        ps_sum = psum_s.tile([P, N5 // P], F32, tag="sums")
        for s in range(N5 // P):
            for c in range(KVC):
                nc.tensor.matmul(
                    ps_sum[:, s : s + 1],
                    lhsT=attnT[:, c, s * P : (s + 1) * P],
                    rhs=ones[:, :1],
                    start=(c == 0),
                    stop=(c == KVC - 1),
                )
        r_s = evac.tile([P, N5 // P], F32, tag="recip")
        nc.vector.reciprocal(r_s, ps_sum)
        return r_s

    def ax_block(attnT):
        axT = attn_pool.tile([P, DC, N5], CDT, tag="axT")  # AX^T: [dv%128, m, q]
        for m in range(DC):
            ps = psum.tile([P, N5], F32, tag="mm")
            for c in range(KVC):
                nc.tensor.matmul(
                    ps,
                    lhsT=xv_s[:, c, m * P : (m + 1) * P],
                    rhs=attnT[:, c, :],
                    start=(c == 0),
                    stop=(c == KVC - 1),
                )
            nc.vector.tensor_copy(out=axT[:, m, :], in_=ps)
        return axT

    def out_block(qb, axT, r_s):
        for s in range(N5 // P):
            for nf in range(D // N5):
                ps = psum.tile([P, N5], F32, tag="mm")
                for m in range(DC):
                    nc.tensor.matmul(
                        ps,
                        lhsT=axT[:, m, s * P : (s + 1) * P],
                        rhs=n2_s[:, m, nf * N5 : (nf + 1) * N5],
                        start=(m == 0),
                        stop=(m == DC - 1),
                    )
                fin = evac.tile([P, N5], F32, tag="fin")
                nc.vector.scalar_tensor_tensor(
                    out=fin,
                    in0=ps,
                    scalar=r_s[:, s : s + 1],
                    in1=bo2_s[:, nf * N5 : (nf + 1) * N5],
                    op0=ALU.mult,
                    op1=ALU.add,
                )
                row0 = qb * N5 + s * P
                nc.sync.dma_start(
                    out=out[row0 : row0 + P, nf * N5 : (nf + 1) * N5], in_=fin
                )

    qprime8(0)
    qprime8(1)
    qpool_cm.__exit__(None, None, None)
    psum = ctx.enter_context(tc.tile_pool(name="psum", bufs=4, space="PSUM"))
    psum_s = ctx.enter_context(tc.tile_pool(name="psum_s", bufs=2, space="PSUM"))
    a0 = scores_exp(0)
    r0 = sums_recip(a0)
    x0 = ax_block(a0)
    out_block(0, x0, r0)
    a1 = scores_exp(1)
    r1 = sums_recip(a1)
    x1 = ax_block(a1)
    out_block(1, x1, r1)


def build_program():
    nc = bacc.Bacc(
        "TRN2", target_bir_lowering=False, debug=False, num_devices=NCORES
    )
    qT = nc.dram_tensor("qT", [D, QL], CDT, kind="ExternalInput").ap()
    kTb = nc.dram_tensor("kTb", [D, KVB], CDT, kind="ExternalInput").ap()
    kT8 = nc.dram_tensor("kT8", [D, SKV - KVB], F8, kind="ExternalInput").ap()
    xv = nc.dram_tensor("xv", [SKV, D], CDT, kind="ExternalInput").ap()
    m32 = nc.dram_tensor("m32", [D, D], CDT, kind="ExternalInput").ap()
    n2 = nc.dram_tensor("n2", [D, D], CDT, kind="ExternalInput").ap()
    t2s = nc.dram_tensor("t2s", [P, KVC], F32, kind="ExternalInput").ap()
    bo2 = nc.dram_tensor("bo2", [D], F32, kind="ExternalInput").ap()
    out = nc.dram_tensor("out", [QL, D], F32, kind="ExternalOutput").ap()

    with tile.TileContext(nc) as tc:
        with ExitStack() as ctx:
            _build_tile(ctx, tc, (qT, kTb, kT8, xv, m32, n2, t2s, bo2, out))
    nc.compile()
    return nc


def prep_in_maps(query, key, value, Wq, bq, Wk, bk, Wv, bv, Wo, bo):
    """Host-side shard prep: fold weights, slice, transpose, cast."""
    query = np.asarray(query, np.float32)
    key = np.asarray(key, np.float32)
    value = np.asarray(value, np.float32)
    Wq = np.asarray(Wq, np.float32)
    Wk = np.asarray(Wk, np.float32)
    Wv = np.asarray(Wv, np.float32)
    Wo = np.asarray(Wo, np.float32)
    bq = np.asarray(bq, np.float32)
    bv = np.asarray(bv, np.float32)
    bo = np.asarray(bo, np.float32)

    M32 = (Wq @ Wk.T) * MS
    N2 = Wv @ Wo
    ck = Wk @ bq  # per-kv score offset direction; zero when bq == 0
    shared = {
        "m32": M32.astype(NP_CDT),
        "n2": N2.astype(NP_CDT),
        "bo2": bv @ Wo + bo,
    }
    in_maps = []
    for b in range(B):
        kT = np.ascontiguousarray(key[b].T)
        kTbb = kT[:, :KVB].astype(NP_CDT)
        kT8b = kT[:, KVB:].astype(NP_F8)
        xvb = value[b].astype(NP_CDT)
        # pre-transposed to [P, KVC] so the DMA is 128 contiguous 64B runs
        # instead of 2048 four-byte runs (descriptor-generation cost)
        t2sb = np.ascontiguousarray(
            (SCALE * (key[b] @ ck)).astype(np.float32).reshape(KVC, P).T
        )
        for h in range(2):
            qTb = np.ascontiguousarray(query[b, h * QL : (h + 1) * QL].T).astype(
                NP_CDT
            )
            in_maps.append(
                {"qT": qTb, "kTb": kTbb, "kT8": kT8b, "xv": xvb, "t2s": t2sb,
                 **shared}
            )
    return in_maps


_NC_CACHE = None


def _get_nc():
    global _NC_CACHE
    if _NC_CACHE is None:
        _NC_CACHE = build_program()
    return _NC_CACHE


def run(inputs, **run_kwargs):
    nc = _get_nc()
    in_maps = prep_in_maps(**inputs)
    res = run_bass_kernel_spmd(nc, in_maps, core_ids=list(range(NCORES)), **run_kwargs)
    out = np.empty((B, SQ, D), np.float32)
    for b in range(B):
        for h in range(2):
            out[b, h * QL : (h + 1) * QL] = res.results[2 * b + h]["out"]
    return out, res


def kernel(query, key, value, Wq, bq, Wk, bk, Wv, bv, Wo, bo):
    out, _ = run(
        dict(
            query=query, key=key, value=value, Wq=Wq, bq=bq, Wk=Wk, bk=bk,
            Wv=Wv, bv=bv, Wo=Wo, bo=bo,
        )
    )
    return out


if __name__ == "__main__":
    rng = np.random.default_rng(0)
    ins = {
        "query": rng.standard_normal((B, SQ, D), dtype=np.float32),
        "key": rng.standard_normal((B, SKV, D), dtype=np.float32),
        "value": rng.standard_normal((B, SKV, D), dtype=np.float32),
        "Wq": (rng.standard_normal((D, D), dtype=np.float32) * 0.02),
        "bq": np.zeros(D, np.float32),
        "Wk": (rng.standard_normal((D, D), dtype=np.float32) * 0.02),
        "bk": np.zeros(D, np.float32),
        "Wv": (rng.standard_normal((D, D), dtype=np.float32) * 0.02),
        "bv": np.zeros(D, np.float32),
        "Wo": (rng.standard_normal((D, D), dtype=np.float32) * 0.02),
        "bo": np.zeros(D, np.float32),
    }
    out = kernel(**ins)
    print("kernel ran, out shape", out.shape)


# revision 34
# speedup vs baseline: 1.0473x; 1.0473x over previous
"""Trainium2 Bass kernel for single-head cross-attention (v3: folded weights).

Reference computation (B=4, Sq=Skv=2048, D=1024, fp32):
    Q = query @ Wq + bq ; K = key @ Wk + bk ; V = value @ Wv + bv
    out = softmax(Q K^T / sqrt(D)) V @ Wo + bo

Since no nonlinearity separates the projections from the score/output
matmuls, the host folds the weights (a static, per-model transform):
    M = Wq @ Wk.T        scores = (query @ M) @ key^T  (K proj eliminated)
    N = Wv @ Wo          out    = (attn @ value) @ N   (V proj eliminated)
Bias terms fold exactly: the per-kv offset key @ (Wk @ bq) becomes the
exp() activation bias; per-q offsets cancel against the softmax
denominator (we divide by the sums at the very end, so they never need
computing); bv @ Wo + bo is the output bias.

This removes 25% of the device FLOPs and, because each core can simply
be HANDED the full raw key/value for its batch, the K/V AllGathers of
v2 disappear entirely. Sharding: 8 shards = (batch b) x (query half h);
core 2*b+h computes output rows [h*1024,(h+1)*1024) of batch b. All
matmul operands are bf16 (fp8 DoubleRow was measured at 1.9e-2 rel err
against the 2e-2 budget - too close).

M is pre-scaled by 32 on the host so Q' = query @ 32M has entries O(15)
(fp32 PSUM doesn't care, but it keeps the bf16 store well-conditioned);
the exp scale absorbs the 2^-10.

Dataflow per core (all contractions land on SBUF partitions):
    Q'^T[e,q]  = M32.T @ qT        (lhsT=m32,  rhs=qT)
    S^T[kv,q]  = key @ Q'^T        (lhsT=kT,   rhs=Q'^T)
    A^T        = exp(S^T/1024 + t2s)            (unnormalized)
    AX^T[dv,q] = value.T @ A^T     (lhsT=xv,   rhs=A^T)
    sums[q,1]  = A @ ones          (lhsT=A^T,  rhs=ones)
    F[q,f]     = AX @ N            (lhsT=AX^T, rhs=n2)
    out        = F * (1/sums) + (bv @ Wo + bo)
"""

import sys

if "/opt/trn_rl_repo" not in sys.path:
    sys.path.insert(0, "/opt/trn_rl_repo")

from contextlib import ExitStack

import ml_dtypes
import numpy as np

import concourse.bass as bass
import concourse.mybir as mybir
import concourse.tile as tile
from concourse import bacc
from concourse.bass_utils import run_bass_kernel_spmd

B, SQ, SKV, D = 4, 2048, 2048, 1024
NCORES = 8
QL = SQ // 2  # local query rows per core
P = 128
DC = D // P  # feature chunks (8)
KVC = SKV // P  # kv chunks (16)
N5 = 512
NQB = QL // N5  # query blocks (2)
F32 = mybir.dt.float32
CDT = mybir.dt.bfloat16
F8 = mybir.dt.float8e4
NP_CDT = ml_dtypes.bfloat16
NP_F8 = ml_dtypes.float8_e4m3fn
MS = 32.0  # host pre-scale on M
SCALE = 1.0 / (32.0 * MS)  # exp scale: 1/sqrt(D) / MS

# kv chunks [KV8C:KVC) compute their scores in fp8e4 DoubleRow (2 k-rows per
# instruction, ~1.8x bf16 rate); softmax attenuates the quantization noise.
# Measured rel err 1.39e-2 at KV8C=8 vs the 2e-2 budget (all-bf16: 3.6e-3).
KV8C = 8  # first fp8 kv chunk; KVC to disable fp8 entirely
KVB = KV8C * P  # kv rows computed in bf16

AF = mybir.ActivationFunctionType
ALU = mybir.AluOpType
DR = mybir.MatmulPerfMode.DoubleRow


def _build_tile(ctx: ExitStack, tc, aps):
    nc = tc.nc
    qT, kTb, kT8, xv, m32, n2, t2s, bo2, out = aps

    big = ctx.enter_context(tc.tile_pool(name="big", bufs=1))
    attn_pool = ctx.enter_context(tc.tile_pool(name="attn", bufs=2))
    evac = ctx.enter_context(tc.tile_pool(name="evac", bufs=4))
    # Q' gets all 8 PSUM banks (one per ec) so each query block is a single
    # dc-outer sweep: the m32 walk then spans 13.9us of matmuls and stays
    # behind the DMA stream. The pool closes before psum/psum_s open.
    qpool_cm = tc.tile_pool(name="qpool", bufs=8, space="PSUM")
    qpool = qpool_cm.__enter__()

    # ---- input DMAs, spread across rings so they stream in parallel --------
    # Critical path: the first Q' psum group consumes every d-chunk of m32,
    # so m32 rides two rings (evens/odds) and qT's first query block leads
    # the third; kT (scores, needed ~15us in) gets the sync ring to itself.
    m32_r = m32.rearrange("(c p) e -> p c e", p=P)
    qT_r = qT.rearrange("(c p) q -> p c q", p=P)
    kTb_r = kTb.rearrange("(c p) n -> p c n", p=P)
    kT8_r = kT8.rearrange("(c p) n -> p c n", p=P)
    xv_r = xv.rearrange("(c p) n -> p c n", p=P)
    n2_r = n2.rearrange("(c p) f -> p c f", p=P)

    # Per-d-chunk DMAs: HWDGE descriptor-generation time is linear in the
    # number of contiguous runs, so one [:, :, slice] DMA costs the same
    # sequencer time as eight [:, c, :] DMAs but delays every consumer until
    # the whole thing is issued. The SDMA engines round-robin between rings
    # that have queued work at packet granularity, so EVERY ring must carry
    # earliest-deadline traffic first: stripe all tensors across the three
    # rings in global consumption order (m32/qT -> kT -> xv -> n2).
    m32_s = big.tile([P, DC, D], CDT, tag="m32")
    qT_s = big.tile([P, DC, QL], CDT, tag="qT")
    kTb_s = big.tile([P, DC, KVB], CDT, tag="kTb")
    kT8_s = big.tile([P, DC, SKV - KVB], F8, tag="kT8")
    xv_s = big.tile([P, KVC, D], CDT, tag="xv")
    n2_s = big.tile([P, DC, D], CDT, tag="n2")
    t2s_s = big.tile([P, KVC], F32, tag="t2s")
    bo2_s = big.tile([P, D], F32, tag="bo2")
    bo2_bcast = bass.AP(tensor=bo2.tensor, offset=bo2.offset, ap=[[0, P], bo2.ap[0]])
    ones = big.tile([P, 1], CDT, tag="ones")
    nc.vector.memset(ones, 1.0)

    # Phases 1-2 (before the exp stream occupies Scalar) use all three rings;
    # later phases avoid the scalar ring so DMA issues never delay the
    # exp/evac activations queued behind them.
    def emit_xfers(xfers, rings):
        for i, (dst, src) in enumerate(xfers):
            rings[i % len(rings)].dma_start(out=dst, in_=src)

    # Phase 1: Q' qb0 operands, in dc consumption order. Later phases are
    # emitted between compute sections (see bottom) so that evac/exp
    # instructions on the scalar queue aren't stuck behind a long run of
    # serialized DIRECT2D descriptor-generation slices.
    xfers = []
    for dc in range(DC):
        xfers.append((m32_s[:, dc, :], m32_r[:, dc, :]))
        xfers.append((qT_s[:, dc, 0:N5], qT_r[:, dc, 0:N5]))
    xfers.append((t2s_s, t2s))  # host ships t2s pre-transposed to [P, KVC]
    emit_xfers(xfers, [nc.sync, nc.scalar, nc.gpsimd])

    def phase2():  # qb1 queries, then scores operands
        xfers = []
        for dc in range(DC):
            xfers.append((qT_s[:, dc, N5:QL], qT_r[:, dc, N5:QL]))
        for dc in range(DC):
            xfers.append((kTb_s[:, dc, :], kTb_r[:, dc, :]))
            xfers.append((kT8_s[:, dc, :], kT8_r[:, dc, :]))
        emit_xfers(xfers, [nc.sync, nc.scalar, nc.gpsimd])

    def phase34():  # AX operand, then output projection operand
        xfers = []
        for c in range(KVC):
            xfers.append((xv_s[:, c, :], xv_r[:, c, :]))
        xfers.append((bo2_s, bo2_bcast))
        for dc in range(DC):
            xfers.append((n2_s[:, dc, :], n2_r[:, dc, :]))
        emit_xfers(xfers, [nc.sync, nc.gpsimd])

    # ---- Q'^T = M32.T @ qT ---------------------------------------------------
    qp = big.tile([P, DC, QL], CDT, tag="qp")  # Q'^T: [d'%128, d'//128, q]
    qp8 = None
    if KV8C < KVC:
        qp8 = big.tile([P, DC, QL], F8, tag="qp8", name="qp8")

    def qp_evac(ec, qb, ps):
        # one evac per psum bank, alternating engines, so the bank-release
        # drain is short; the fp8 copy for the DoubleRow scores is derived
        # from qp lazily (qp8_fill), off the bank-critical path.
        sl = slice(qb * N5, (qb + 1) * N5)
        if ec % 2 == 0:
            nc.scalar.activation(
                out=qp[:, ec, sl], in_=ps, func=AF.Identity, scale=1.0
            )
        else:
            nc.vector.tensor_copy(out=qp[:, ec, sl], in_=ps)

    def qp8_fill(qb):
        if qp8 is None:
            return
        sl = slice(qb * N5, (qb + 1) * N5)
        for ec in range(DC):
            nc.vector.tensor_copy(out=qp8[:, ec, sl], in_=qp[:, ec, sl])

    def qprime8(qb):
        pss = [
            qpool.tile([P, N5], F32, tag="qmm", name=f"qps{qb}_{ec}")
            for ec in range(DC)
        ]
        for dc in range(DC):
            for ec in range(DC):
                nc.tensor.matmul(
                    pss[ec],
                    lhsT=m32_s[:, dc, ec * P : (ec + 1) * P],
                    rhs=qT_s[:, dc, qb * N5 : (qb + 1) * N5],
                    start=(dc == 0),
                    stop=(dc == DC - 1),
                )
        for ec in range(DC):
            qp_evac(ec, qb, pss[ec])

    # ---- scores + exp + sums + AX, one kv pass per 512-query block ----------
    def scores_exp(qb):
        attnT = attn_pool.tile([P, KVC, N5], CDT, tag="attnT")
        qsl = slice(qb * N5, (qb + 1) * N5)
        for c in range(KVC):
            ps = psum.tile([P, N5], F32, tag="mm")
            if c < KV8C:  # bf16 path
                for dc in range(DC):
                    nc.tensor.matmul(
                        ps,
                        lhsT=kTb_s[:, dc, c * P : (c + 1) * P],
                        rhs=qp[:, dc, qsl],
                        start=(dc == 0),
                        stop=(dc == DC - 1),
                    )
            else:  # fp8 DoubleRow path: two d-chunks per instruction
                c8 = c - KV8C
                for dc in range(0, DC, 2):
                    nc.tensor.matmul(
                        ps,
                        lhsT=kT8_s[:, dc : dc + 2, c8 * P : (c8 + 1) * P],
                        rhs=qp8[:, dc : dc + 2, qsl],
                        start=(dc == 0),
                        stop=(dc == DC - 2),
                        perf_mode=DR,
                    )
            nc.scalar.activation(
                out=attnT[:, c, :],
                in_=ps,
                func=AF.Exp,
                scale=SCALE,
                bias=t2s_s[:, c : c + 1],
            )
        return attnT

    def sums_recip(attnT):
        ps_sum = psum_s.tile([P, N5 // P], F32, tag="sums")
        for s in range(N5 // P):
            for c in range(KVC):
                nc.tensor.matmul(
                    ps_sum[:, s : s + 1],
                    lhsT=attnT[:, c, s * P : (s + 1) * P],
                    rhs=ones[:, :1],
                    start=(c == 0),
                    stop=(c == KVC - 1),
                )
        r_s = evac.tile([P, N5 // P], F32, tag="recip")
        nc.vector.reciprocal(r_s, ps_sum)
        return r_s

    def ax_block(attnT):
        axT = attn_pool.tile([P, DC, N5], CDT, tag="axT")  # AX^T: [dv%128, m, q]
        for m in range(DC):
            ps = psum.tile([P, N5], F32, tag="mm")
            for c in range(KVC):
                nc.tensor.matmul(
                    ps,
                    lhsT=xv_s[:, c, m * P : (m + 1) * P],
                    rhs=attnT[:, c, :],
                    start=(c == 0),
                    stop=(c == KVC - 1),
                )
            nc.vector.tensor_copy(out=axT[:, m, :], in_=ps)
        return axT

    def out_block(qb, axT, r_s):
        for s in range(N5 // P):
            for nf in range(D // N5):
                ps = psum.tile([P, N5], F32, tag="mm")
                for m in range(DC):
                    nc.tensor.matmul(
                        ps,
                        lhsT=axT[:, m, s * P : (s + 1) * P],
                        rhs=n2_s[:, m, nf * N5 : (nf + 1) * N5],
                        start=(m == 0),
                        stop=(m == DC - 1),
                    )
                fin = evac.tile([P, N5], F32, tag="fin")
                nc.vector.scalar_tensor_tensor(
                    out=fin,
                    in0=ps,
                    scalar=r_s[:, s : s + 1],
                    in1=bo2_s[:, nf * N5 : (nf + 1) * N5],
                    op0=ALU.mult,
                    op1=ALU.add,
                )
                row0 = qb * N5 + s * P
                nc.sync.dma_start(
                    out=out[row0 : row0 + P, nf * N5 : (nf + 1) * N5], in_=fin
                )

    qprime8(0)
    phase2()
    qp8_fill(0)
    qprime8(1)
    phase34()
    qp8_fill(1)
    qpool_cm.__exit__(None, None, None)
    psum = ctx.enter_context(tc.tile_pool(name="psum", bufs=4, space="PSUM"))
    psum_s = ctx.enter_context(tc.tile_pool(name="psum_s", bufs=2, space="PSUM"))
    a0 = scores_exp(0)
    r0 = sums_recip(a0)
    x0 = ax_block(a0)
    out_block(0, x0, r0)
    a1 = scores_exp(1)
    r1 = sums_recip(a1)
    x1 = ax_block(a1)
    out_block(1, x1, r1)


def build_program():
    nc = bacc.Bacc(
        "TRN2", target_bir_lowering=False, debug=False, num_devices=NCORES
    )
    qT = nc.dram_tensor("qT", [D, QL], CDT, kind="ExternalInput").ap()
    kTb = nc.dram_tensor("kTb", [D, KVB], CDT, kind="ExternalInput").ap()
    kT8 = nc.dram_tensor("kT8", [D, SKV - KVB], F8, kind="ExternalInput").ap()
    xv = nc.dram_tensor("xv", [SKV, D], CDT, kind="ExternalInput").ap()
    m32 = nc.dram_tensor("m32", [D, D], CDT, kind="ExternalInput").ap()
    n2 = nc.dram_tensor("n2", [D, D], CDT, kind="ExternalInput").ap()
    t2s = nc.dram_tensor("t2s", [P, KVC], F32, kind="ExternalInput").ap()
    bo2 = nc.dram_tensor("bo2", [D], F32, kind="ExternalInput").ap()
    out = nc.dram_tensor("out", [QL, D], F32, kind="ExternalOutput").ap()

    with tile.TileContext(nc) as tc:
        with ExitStack() as ctx:
            _build_tile(ctx, tc, (qT, kTb, kT8, xv, m32, n2, t2s, bo2, out))
    nc.compile()
    return nc


def prep_in_maps(query, key, value, Wq, bq, Wk, bk, Wv, bv, Wo, bo):
    """Host-side shard prep: fold weights, slice, transpose, cast."""
    query = np.asarray(query, np.float32)
    key = np.asarray(key, np.float32)
    value = np.asarray(value, np.float32)
    Wq = np.asarray(Wq, np.float32)
    Wk = np.asarray(Wk, np.float32)
    Wv = np.asarray(Wv, np.float32)
    Wo = np.asarray(Wo, np.float32)
    bq = np.asarray(bq, np.float32)
    bv = np.asarray(bv, np.float32)
    bo = np.asarray(bo, np.float32)

    M32 = (Wq @ Wk.T) * MS
    N2 = Wv @ Wo
    ck = Wk @ bq  # per-kv score offset direction; zero when bq == 0
    shared = {
        "m32": M32.astype(NP_CDT),
        "n2": N2.astype(NP_CDT),
        "bo2": bv @ Wo + bo,
    }
    in_maps = []
    for b in range(B):
        kT = np.ascontiguousarray(key[b].T)
        kTbb = kT[:, :KVB].astype(NP_CDT)
        kT8b = kT[:, KVB:].astype(NP_F8)
        xvb = value[b].astype(NP_CDT)
        # pre-transposed to [P, KVC] so the DMA is 128 contiguous 64B runs
        # instead of 2048 four-byte runs (descriptor-generation cost)
        t2sb = np.ascontiguousarray(
            (SCALE * (key[b] @ ck)).astype(np.float32).reshape(KVC, P).T
        )
        for h in range(2):
            qTb = np.ascontiguousarray(query[b, h * QL : (h + 1) * QL].T).astype(
                NP_CDT
            )
            in_maps.append(
                {"qT": qTb, "kTb": kTbb, "kT8": kT8b, "xv": xvb, "t2s": t2sb,
                 **shared}
            )
    return in_maps


_NC_CACHE = None


def _get_nc():
    global _NC_CACHE
    if _NC_CACHE is None:
        _NC_CACHE = build_program()
    return _NC_CACHE


def run(inputs, **run_kwargs):
    nc = _get_nc()
    in_maps = prep_in_maps(**inputs)
    res = run_bass_kernel_spmd(nc, in_maps, core_ids=list(range(NCORES)), **run_kwargs)
    out = np.empty((B, SQ, D), np.float32)
    for b in range(B):
        for h in range(2):
            out[b, h * QL : (h + 1) * QL] = res.results[2 * b + h]["out"]
    return out, res


def kernel(query, key, value, Wq, bq, Wk, bk, Wv, bv, Wo, bo):
    out, _ = run(
        dict(
            query=query, key=key, value=value, Wq=Wq, bq=bq, Wk=Wk, bk=bk,
            Wv=Wv, bv=bv, Wo=Wo, bo=bo,
        )
    )
    return out


if __name__ == "__main__":
    rng = np.random.default_rng(0)
    ins = {
        "query": rng.standard_normal((B, SQ, D), dtype=np.float32),
        "key": rng.standard_normal((B, SKV, D), dtype=np.float32),
        "value": rng.standard_normal((B, SKV, D), dtype=np.float32),
        "Wq": (rng.standard_normal((D, D), dtype=np.float32) * 0.02),
        "bq": np.zeros(D, np.float32),
        "Wk": (rng.standard_normal((D, D), dtype=np.float32) * 0.02),
        "bk": np.zeros(D, np.float32),
        "Wv": (rng.standard_normal((D, D), dtype=np.float32) * 0.02),
        "bv": np.zeros(D, np.float32),
        "Wo": (rng.standard_normal((D, D), dtype=np.float32) * 0.02),
        "bo": np.zeros(D, np.float32),
    }
    out = kernel(**ins)
    print("kernel ran, out shape", out.shape)


# revision 35
# speedup vs baseline: 1.0627x; 1.0147x over previous
"""Trainium2 Bass kernel for single-head cross-attention (v3: folded weights).

Reference computation (B=4, Sq=Skv=2048, D=1024, fp32):
    Q = query @ Wq + bq ; K = key @ Wk + bk ; V = value @ Wv + bv
    out = softmax(Q K^T / sqrt(D)) V @ Wo + bo

Since no nonlinearity separates the projections from the score/output
matmuls, the host folds the weights (a static, per-model transform):
    M = Wq @ Wk.T        scores = (query @ M) @ key^T  (K proj eliminated)
    N = Wv @ Wo          out    = (attn @ value) @ N   (V proj eliminated)
Bias terms fold exactly: the per-kv offset key @ (Wk @ bq) becomes the
exp() activation bias; per-q offsets cancel against the softmax
denominator (we divide by the sums at the very end, so they never need
computing); bv @ Wo + bo is the output bias.

This removes 25% of the device FLOPs and, because each core can simply
be HANDED the full raw key/value for its batch, the K/V AllGathers of
v2 disappear entirely. Sharding: 8 shards = (batch b) x (query half h);
core 2*b+h computes output rows [h*1024,(h+1)*1024) of batch b. All
matmul operands are bf16 (fp8 DoubleRow was measured at 1.9e-2 rel err
against the 2e-2 budget - too close).

M is pre-scaled by 32 on the host so Q' = query @ 32M has entries O(15)
(fp32 PSUM doesn't care, but it keeps the bf16 store well-conditioned);
the exp scale absorbs the 2^-10.

Dataflow per core (all contractions land on SBUF partitions):
    Q'^T[e,q]  = M32.T @ qT        (lhsT=m32,  rhs=qT)
    S^T[kv,q]  = key @ Q'^T        (lhsT=kT,   rhs=Q'^T)
    A^T        = exp(S^T/1024 + t2s)            (unnormalized)
    AX^T[dv,q] = value.T @ A^T     (lhsT=xv,   rhs=A^T)
    sums[q,1]  = A @ ones          (lhsT=A^T,  rhs=ones)
    F[q,f]     = AX @ N            (lhsT=AX^T, rhs=n2)
    out        = F * (1/sums) + (bv @ Wo + bo)
"""

import sys

if "/opt/trn_rl_repo" not in sys.path:
    sys.path.insert(0, "/opt/trn_rl_repo")

from contextlib import ExitStack

import ml_dtypes
import numpy as np

import concourse.bass as bass
import concourse.mybir as mybir
import concourse.tile as tile
from concourse import bacc
from concourse.bass_utils import run_bass_kernel_spmd

B, SQ, SKV, D = 4, 2048, 2048, 1024
NCORES = 8
QL = SQ // 2  # local query rows per core
P = 128
DC = D // P  # feature chunks (8)
KVC = SKV // P  # kv chunks (16)
N5 = 512
NQB = QL // N5  # query blocks (2)
F32 = mybir.dt.float32
CDT = mybir.dt.bfloat16
F8 = mybir.dt.float8e4
NP_CDT = ml_dtypes.bfloat16
NP_F8 = ml_dtypes.float8_e4m3fn
MS = 32.0  # host pre-scale on M
SCALE = 1.0 / (32.0 * MS)  # exp scale: 1/sqrt(D) / MS

# kv chunks [KV8C:KVC) compute their scores in fp8e4 DoubleRow (2 k-rows per
# instruction, ~1.8x bf16 rate); softmax attenuates the quantization noise.
# Measured rel err 1.39e-2 at KV8C=8 vs the 2e-2 budget (all-bf16: 3.6e-3).
KV8C = 8  # first fp8 kv chunk; KVC to disable fp8 entirely
KVB = KV8C * P  # kv rows computed in bf16

AF = mybir.ActivationFunctionType
ALU = mybir.AluOpType
DR = mybir.MatmulPerfMode.DoubleRow


def _build_tile(ctx: ExitStack, tc, aps):
    nc = tc.nc
    qT, kTb, kT8, xv, m32, n2, t2s, bo2, out = aps

    big = ctx.enter_context(tc.tile_pool(name="big", bufs=1))
    attn_pool = ctx.enter_context(tc.tile_pool(name="attn", bufs=2))
    evac = ctx.enter_context(tc.tile_pool(name="evac", bufs=4))
    # Q' gets all 8 PSUM banks (one per ec) so each query block is a single
    # dc-outer sweep: the m32 walk then spans 13.9us of matmuls and stays
    # behind the DMA stream. The pool closes before psum/psum_s open.
    qpool_cm = tc.tile_pool(name="qpool", bufs=8, space="PSUM")
    qpool = qpool_cm.__enter__()

    # ---- input DMAs, spread across rings so they stream in parallel --------
    # Critical path: the first Q' psum group consumes every d-chunk of m32,
    # so m32 rides two rings (evens/odds) and qT's first query block leads
    # the third; kT (scores, needed ~15us in) gets the sync ring to itself.
    m32_r = m32.rearrange("(c p) e -> p c e", p=P)
    qT_r = qT.rearrange("(c p) q -> p c q", p=P)
    kTb_r = kTb.rearrange("(c p) n -> p c n", p=P)
    kT8_r = kT8.rearrange("(c p) n -> p c n", p=P)
    xv_r = xv.rearrange("(c p) n -> p c n", p=P)
    n2_r = n2.rearrange("(c p) f -> p c f", p=P)

    # Per-d-chunk DMAs: HWDGE descriptor-generation time is linear in the
    # number of contiguous runs, so one [:, :, slice] DMA costs the same
    # sequencer time as eight [:, c, :] DMAs but delays every consumer until
    # the whole thing is issued. The SDMA engines round-robin between rings
    # that have queued work at packet granularity, so EVERY ring must carry
    # earliest-deadline traffic first: stripe all tensors across the three
    # rings in global consumption order (m32/qT -> kT -> xv -> n2).
    m32_s = big.tile([P, DC, D], CDT, tag="m32")
    qT_s = big.tile([P, DC, QL], CDT, tag="qT")
    kTb_s = big.tile([P, DC, KVB], CDT, tag="kTb")
    kT8_s = big.tile([P, DC, SKV - KVB], F8, tag="kT8")
    xv_s = big.tile([P, KVC, D], CDT, tag="xv")
    n2_s = big.tile([P, DC, D], CDT, tag="n2")
    t2s_s = big.tile([P, KVC], F32, tag="t2s")
    bo2_s = big.tile([P, D], F32, tag="bo2")
    bo2_bcast = bass.AP(tensor=bo2.tensor, offset=bo2.offset, ap=[[0, P], bo2.ap[0]])
    ones = big.tile([P, 1], CDT, tag="ones")
    nc.vector.memset(ones, 1.0)

    # Phases 1-2 (before the exp stream occupies Scalar) use all three rings;
    # later phases avoid the scalar ring so DMA issues never delay the
    # exp/evac activations queued behind them.
    def emit_xfers(xfers, rings):
        for i, (dst, src) in enumerate(xfers):
            rings[i % len(rings)].dma_start(out=dst, in_=src)

    # Phase 1: Q' qb0 operands, in dc consumption order. Later phases are
    # emitted between compute sections (see bottom) so that evac/exp
    # instructions on the scalar queue aren't stuck behind a long run of
    # serialized DIRECT2D descriptor-generation slices.
    # ~512KB per DMA: each ring item pays ~1.5us of fixed issue+completion
    # latency, so smaller chunks cap per-ring throughput well below HBM BW.
    xfers = []
    for dc in range(0, DC, 2):
        xfers.append((m32_s[:, dc : dc + 2, :], m32_r[:, dc : dc + 2, :]))
        xfers.append(
            (qT_s[:, dc : dc + 2, 0:QL], qT_r[:, dc : dc + 2, 0:QL])
        )
    xfers.append((t2s_s, t2s))  # host ships t2s pre-transposed to [P, KVC]
    emit_xfers(xfers, [nc.sync, nc.scalar, nc.gpsimd])

    def phase2():  # scores operands (qT now fully loaded in phase 1)
        xfers = []
        for dc in range(0, DC, 2):
            xfers.append((kTb_s[:, dc : dc + 2, :], kTb_r[:, dc : dc + 2, :]))
        for dc in range(0, DC, 4):
            xfers.append((kT8_s[:, dc : dc + 4, :], kT8_r[:, dc : dc + 4, :]))
        emit_xfers(xfers, [nc.sync, nc.scalar, nc.gpsimd])

    def phase34():  # AX operand, then output projection operand
        xfers = []
        for c in range(0, KVC, 2):
            xfers.append((xv_s[:, c : c + 2, :], xv_r[:, c : c + 2, :]))
        xfers.append((bo2_s, bo2_bcast))
        for dc in range(0, DC, 2):
            xfers.append((n2_s[:, dc : dc + 2, :], n2_r[:, dc : dc + 2, :]))
        emit_xfers(xfers, [nc.sync, nc.gpsimd])

    # ---- Q'^T = M32.T @ qT ---------------------------------------------------
    qp = big.tile([P, DC, QL], CDT, tag="qp")  # Q'^T: [d'%128, d'//128, q]
    qp8 = None
    if KV8C < KVC:
        qp8 = big.tile([P, DC, QL], F8, tag="qp8", name="qp8")

    def qp_evac(ec, qb, ps):
        # one evac per psum bank, alternating engines, so the bank-release
        # drain is short; the fp8 copy for the DoubleRow scores is derived
        # from qp lazily (qp8_fill), off the bank-critical path.
        sl = slice(qb * N5, (qb + 1) * N5)
        if ec % 2 == 0:
            nc.scalar.activation(
                out=qp[:, ec, sl], in_=ps, func=AF.Identity, scale=1.0
            )
        else:
            nc.vector.tensor_copy(out=qp[:, ec, sl], in_=ps)

    def qp8_fill(qb):
        if qp8 is None:
            return
        sl = slice(qb * N5, (qb + 1) * N5)
        for ec in range(DC):
            nc.vector.tensor_copy(out=qp8[:, ec, sl], in_=qp[:, ec, sl])

    def qprime8(qb):
        pss = [
            qpool.tile([P, N5], F32, tag="qmm", name=f"qps{qb}_{ec}")
            for ec in range(DC)
        ]
        for dc in range(DC):
            for ec in range(DC):
                nc.tensor.matmul(
                    pss[ec],
                    lhsT=m32_s[:, dc, ec * P : (ec + 1) * P],
                    rhs=qT_s[:, dc, qb * N5 : (qb + 1) * N5],
                    start=(dc == 0),
                    stop=(dc == DC - 1),
                )
        for ec in range(DC):
            qp_evac(ec, qb, pss[ec])

    # ---- scores + exp + sums + AX, one kv pass per 512-query block ----------
    def scores_exp(qb):
        attnT = attn_pool.tile([P, KVC, N5], CDT, tag="attnT")
        qsl = slice(qb * N5, (qb + 1) * N5)
        for c in range(KVC):
            ps = psum.tile([P, N5], F32, tag="mm")
            if c < KV8C:  # bf16 path
                for dc in range(DC):
                    nc.tensor.matmul(
                        ps,
                        lhsT=kTb_s[:, dc, c * P : (c + 1) * P],
                        rhs=qp[:, dc, qsl],
                        start=(dc == 0),
                        stop=(dc == DC - 1),
                    )
            else:  # fp8 DoubleRow path: two d-chunks per instruction
                c8 = c - KV8C
                for dc in range(0, DC, 2):
                    nc.tensor.matmul(
                        ps,
                        lhsT=kT8_s[:, dc : dc + 2, c8 * P : (c8 + 1) * P],
                        rhs=qp8[:, dc : dc + 2, qsl],
                        start=(dc == 0),
                        stop=(dc == DC - 2),
                        perf_mode=DR,
                    )
            nc.scalar.activation(
                out=attnT[:, c, :],
                in_=ps,
                func=AF.Exp,
                scale=SCALE,
                bias=t2s_s[:, c : c + 1],
            )
        return attnT

    def sums_recip(attnT):
        ps_sum = psum_s.tile([P, N5 // P], F32, tag="sums")
        for s in range(N5 // P):
            for c in range(KVC):
                nc.tensor.matmul(
                    ps_sum[:, s : s + 1],
                    lhsT=attnT[:, c, s * P : (s + 1) * P],
                    rhs=ones[:, :1],
                    start=(c == 0),
                    stop=(c == KVC - 1),
                )
        r_s = evac.tile([P, N5 // P], F32, tag="recip")
        nc.vector.reciprocal(r_s, ps_sum)
        return r_s

    def ax_block(attnT):
        axT = attn_pool.tile([P, DC, N5], CDT, tag="axT")  # AX^T: [dv%128, m, q]
        for m in range(DC):
            ps = psum.tile([P, N5], F32, tag="mm")
            for c in range(KVC):
                nc.tensor.matmul(
                    ps,
                    lhsT=xv_s[:, c, m * P : (m + 1) * P],
                    rhs=attnT[:, c, :],
                    start=(c == 0),
                    stop=(c == KVC - 1),
                )
            nc.vector.tensor_copy(out=axT[:, m, :], in_=ps)
        return axT

    def out_block(qb, axT, r_s):
        for s in range(N5 // P):
            for nf in range(D // N5):
                ps = psum.tile([P, N5], F32, tag="mm")
                for m in range(DC):
                    nc.tensor.matmul(
                        ps,
                        lhsT=axT[:, m, s * P : (s + 1) * P],
                        rhs=n2_s[:, m, nf * N5 : (nf + 1) * N5],
                        start=(m == 0),
                        stop=(m == DC - 1),
                    )
                fin = evac.tile([P, N5], F32, tag="fin")
                nc.vector.scalar_tensor_tensor(
                    out=fin,
                    in0=ps,
                    scalar=r_s[:, s : s + 1],
                    in1=bo2_s[:, nf * N5 : (nf + 1) * N5],
                    op0=ALU.mult,
                    op1=ALU.add,
                )
                row0 = qb * N5 + s * P
                nc.sync.dma_start(
                    out=out[row0 : row0 + P, nf * N5 : (nf + 1) * N5], in_=fin
                )

    qprime8(0)
    phase2()
    qp8_fill(0)
    qprime8(1)
    phase34()
    qp8_fill(1)
    qpool_cm.__exit__(None, None, None)
    psum = ctx.enter_context(tc.tile_pool(name="psum", bufs=4, space="PSUM"))
    psum_s = ctx.enter_context(tc.tile_pool(name="psum_s", bufs=2, space="PSUM"))
    a0 = scores_exp(0)
    r0 = sums_recip(a0)
    x0 = ax_block(a0)
    out_block(0, x0, r0)
    a1 = scores_exp(1)
    r1 = sums_recip(a1)
    x1 = ax_block(a1)
    out_block(1, x1, r1)


def build_program():
    nc = bacc.Bacc(
        "TRN2", target_bir_lowering=False, debug=False, num_devices=NCORES
    )
    qT = nc.dram_tensor("qT", [D, QL], CDT, kind="ExternalInput").ap()
    kTb = nc.dram_tensor("kTb", [D, KVB], CDT, kind="ExternalInput").ap()
    kT8 = nc.dram_tensor("kT8", [D, SKV - KVB], F8, kind="ExternalInput").ap()
    xv = nc.dram_tensor("xv", [SKV, D], CDT, kind="ExternalInput").ap()
    m32 = nc.dram_tensor("m32", [D, D], CDT, kind="ExternalInput").ap()
    n2 = nc.dram_tensor("n2", [D, D], CDT, kind="ExternalInput").ap()
    t2s = nc.dram_tensor("t2s", [P, KVC], F32, kind="ExternalInput").ap()
    bo2 = nc.dram_tensor("bo2", [D], F32, kind="ExternalInput").ap()
    out = nc.dram_tensor("out", [QL, D], F32, kind="ExternalOutput").ap()

    with tile.TileContext(nc) as tc:
        with ExitStack() as ctx:
            _build_tile(ctx, tc, (qT, kTb, kT8, xv, m32, n2, t2s, bo2, out))
    nc.compile()
    return nc


def prep_in_maps(query, key, value, Wq, bq, Wk, bk, Wv, bv, Wo, bo):
    """Host-side shard prep: fold weights, slice, transpose, cast."""
    query = np.asarray(query, np.float32)
    key = np.asarray(key, np.float32)
    value = np.asarray(value, np.float32)
    Wq = np.asarray(Wq, np.float32)
    Wk = np.asarray(Wk, np.float32)
    Wv = np.asarray(Wv, np.float32)
    Wo = np.asarray(Wo, np.float32)
    bq = np.asarray(bq, np.float32)
    bv = np.asarray(bv, np.float32)
    bo = np.asarray(bo, np.float32)

    M32 = (Wq @ Wk.T) * MS
    N2 = Wv @ Wo
    ck = Wk @ bq  # per-kv score offset direction; zero when bq == 0
    shared = {
        "m32": M32.astype(NP_CDT),
        "n2": N2.astype(NP_CDT),
        "bo2": bv @ Wo + bo,
    }
    in_maps = []
    for b in range(B):
        kT = np.ascontiguousarray(key[b].T)
        kTbb = kT[:, :KVB].astype(NP_CDT)
        kT8b = kT[:, KVB:].astype(NP_F8)
        xvb = value[b].astype(NP_CDT)
        # pre-transposed to [P, KVC] so the DMA is 128 contiguous 64B runs
        # instead of 2048 four-byte runs (descriptor-generation cost)
        t2sb = np.ascontiguousarray(
            (SCALE * (key[b] @ ck)).astype(np.float32).reshape(KVC, P).T
        )
        for h in range(2):
            qTb = np.ascontiguousarray(query[b, h * QL : (h + 1) * QL].T).astype(
                NP_CDT
            )
            in_maps.append(
                {"qT": qTb, "kTb": kTbb, "kT8": kT8b, "xv": xvb, "t2s": t2sb,
                 **shared}
            )
    return in_maps


_NC_CACHE = None


def _get_nc():
    global _NC_CACHE
    if _NC_CACHE is None:
        _NC_CACHE = build_program()
    return _NC_CACHE


def run(inputs, **run_kwargs):
    nc = _get_nc()
    in_maps = prep_in_maps(**inputs)
    res = run_bass_kernel_spmd(nc, in_maps, core_ids=list(range(NCORES)), **run_kwargs)
    out = np.empty((B, SQ, D), np.float32)
    for b in range(B):
        for h in range(2):
            out[b, h * QL : (h + 1) * QL] = res.results[2 * b + h]["out"]
    return out, res


def kernel(query, key, value, Wq, bq, Wk, bk, Wv, bv, Wo, bo):
    out, _ = run(
        dict(
            query=query, key=key, value=value, Wq=Wq, bq=bq, Wk=Wk, bk=bk,
            Wv=Wv, bv=bv, Wo=Wo, bo=bo,
        )
    )
    return out


if __name__ == "__main__":
    rng = np.random.default_rng(0)
    ins = {
        "query": rng.standard_normal((B, SQ, D), dtype=np.float32),
        "key": rng.standard_normal((B, SKV, D), dtype=np.float32),
        "value": rng.standard_normal((B, SKV, D), dtype=np.float32),
        "Wq": (rng.standard_normal((D, D), dtype=np.float32) * 0.02),
        "bq": np.zeros(D, np.float32),
        "Wk": (rng.standard_normal((D, D), dtype=np.float32) * 0.02),
        "bk": np.zeros(D, np.float32),
        "Wv": (rng.standard_normal((D, D), dtype=np.float32) * 0.02),
        "bv": np.zeros(D, np.float32),
        "Wo": (rng.standard_normal((D, D), dtype=np.float32) * 0.02),
        "bo": np.zeros(D, np.float32),
    }
    out = kernel(**ins)
    print("kernel ran, out shape", out.shape)


# revision 36
# speedup vs baseline: 1.0891x; 1.0248x over previous
"""Trainium2 Bass kernel for single-head cross-attention (v3: folded weights).

Reference computation (B=4, Sq=Skv=2048, D=1024, fp32):
    Q = query @ Wq + bq ; K = key @ Wk + bk ; V = value @ Wv + bv
    out = softmax(Q K^T / sqrt(D)) V @ Wo + bo

Since no nonlinearity separates the projections from the score/output
matmuls, the host folds the weights (a static, per-model transform):
    M = Wq @ Wk.T        scores = (query @ M) @ key^T  (K proj eliminated)
    N = Wv @ Wo          out    = (attn @ value) @ N   (V proj eliminated)
Bias terms fold exactly: the per-kv offset key @ (Wk @ bq) becomes the
exp() activation bias; per-q offsets cancel against the softmax
denominator (we divide by the sums at the very end, so they never need
computing); bv @ Wo + bo is the output bias.

This removes 25% of the device FLOPs and, because each core can simply
be HANDED the full raw key/value for its batch, the K/V AllGathers of
v2 disappear entirely. Sharding: 8 shards = (batch b) x (query half h);
core 2*b+h computes output rows [h*1024,(h+1)*1024) of batch b. All
matmul operands are bf16 (fp8 DoubleRow was measured at 1.9e-2 rel err
against the 2e-2 budget - too close).

M is pre-scaled by 32 on the host so Q' = query @ 32M has entries O(15)
(fp32 PSUM doesn't care, but it keeps the bf16 store well-conditioned);
the exp scale absorbs the 2^-10.

Dataflow per core (all contractions land on SBUF partitions):
    Q'^T[e,q]  = M32.T @ qT        (lhsT=m32,  rhs=qT)
    S^T[kv,q]  = key @ Q'^T        (lhsT=kT,   rhs=Q'^T)
    A^T        = exp(S^T/1024 + t2s)            (unnormalized)
    AX^T[dv,q] = value.T @ A^T     (lhsT=xv,   rhs=A^T)
    sums[q,1]  = A @ ones          (lhsT=A^T,  rhs=ones)
    F[q,f]     = AX @ N            (lhsT=AX^T, rhs=n2)
    out        = F * (1/sums) + (bv @ Wo + bo)
"""

import sys

if "/opt/trn_rl_repo" not in sys.path:
    sys.path.insert(0, "/opt/trn_rl_repo")

from contextlib import ExitStack

import ml_dtypes
import numpy as np

import concourse.bass as bass
import concourse.mybir as mybir
import concourse.tile as tile
from concourse import bacc
from concourse.bass_utils import run_bass_kernel_spmd

B, SQ, SKV, D = 4, 2048, 2048, 1024
NCORES = 8
QL = SQ // 2  # local query rows per core
P = 128
DC = D // P  # feature chunks (8)
KVC = SKV // P  # kv chunks (16)
N5 = 512
NQB = QL // N5  # query blocks (2)
F32 = mybir.dt.float32
CDT = mybir.dt.bfloat16
F8 = mybir.dt.float8e4
NP_CDT = ml_dtypes.bfloat16
NP_F8 = ml_dtypes.float8_e4m3fn
MS = 32.0  # host pre-scale on M
SCALE = 1.0 / (32.0 * MS)  # exp scale: 1/sqrt(D) / MS

# kv chunks [KV8C:KVC) compute their scores in fp8e4 DoubleRow (2 k-rows per
# instruction, ~1.8x bf16 rate); softmax attenuates the quantization noise.
# Measured rel err 1.39e-2 at KV8C=8 vs the 2e-2 budget (all-bf16: 3.6e-3).
KV8C = 8  # first fp8 kv chunk; KVC to disable fp8 entirely
KVB = KV8C * P  # kv rows computed in bf16

AF = mybir.ActivationFunctionType
ALU = mybir.AluOpType
DR = mybir.MatmulPerfMode.DoubleRow


def _build_tile(ctx: ExitStack, tc, aps):
    nc = tc.nc
    qT, kTb, kT8, xv, m32, n2, t2s, bo2, out = aps

    big = ctx.enter_context(tc.tile_pool(name="big", bufs=1))
    attn_pool = ctx.enter_context(tc.tile_pool(name="attn", bufs=2))
    evac = ctx.enter_context(tc.tile_pool(name="evac", bufs=4))
    # Q' gets all 8 PSUM banks (one per ec) so each query block is a single
    # dc-outer sweep: the m32 walk then spans 13.9us of matmuls and stays
    # behind the DMA stream. The pool closes before psum/psum_s open.
    qpool_cm = tc.tile_pool(name="qpool", bufs=8, space="PSUM")
    qpool = qpool_cm.__enter__()

    # ---- input DMAs, spread across rings so they stream in parallel --------
    # Critical path: the first Q' psum group consumes every d-chunk of m32,
    # so m32 rides two rings (evens/odds) and qT's first query block leads
    # the third; kT (scores, needed ~15us in) gets the sync ring to itself.
    m32_r = m32.rearrange("(c p) e -> p c e", p=P)
    qT_r = qT.rearrange("(c p) q -> p c q", p=P)
    kTb_r = kTb.rearrange("(c p) n -> p c n", p=P)
    kT8_r = kT8.rearrange("(c p) n -> p c n", p=P)
    xv_r = xv.rearrange("(c p) n -> p c n", p=P)
    n2_r = n2.rearrange("(c p) f -> p c f", p=P)

    # Per-d-chunk DMAs: HWDGE descriptor-generation time is linear in the
    # number of contiguous runs, so one [:, :, slice] DMA costs the same
    # sequencer time as eight [:, c, :] DMAs but delays every consumer until
    # the whole thing is issued. The SDMA engines round-robin between rings
    # that have queued work at packet granularity, so EVERY ring must carry
    # earliest-deadline traffic first: stripe all tensors across the three
    # rings in global consumption order (m32/qT -> kT -> xv -> n2).
    m32_s = big.tile([P, DC, D], CDT, tag="m32")
    qT_s = big.tile([P, DC, QL], CDT, tag="qT")
    kTb_s = big.tile([P, DC, KVB], CDT, tag="kTb")
    kT8_s = big.tile([P, DC, SKV - KVB], F8, tag="kT8")
    xv_s = big.tile([P, KVC, D], CDT, tag="xv")
    n2_s = big.tile([P, DC, D], CDT, tag="n2")
    t2s_s = big.tile([P, KVC], F32, tag="t2s")
    bo2_s = big.tile([P, D], F32, tag="bo2")
    bo2_bcast = bass.AP(tensor=bo2.tensor, offset=bo2.offset, ap=[[0, P], bo2.ap[0]])
    ones = big.tile([P, 1], CDT, tag="ones")
    nc.vector.memset(ones, 1.0)

    # Phases 1-2 (before the exp stream occupies Scalar) use all three rings;
    # later phases avoid the scalar ring so DMA issues never delay the
    # exp/evac activations queued behind them.
    def emit_xfers(xfers, rings):
        for i, (dst, src) in enumerate(xfers):
            rings[i % len(rings)].dma_start(out=dst, in_=src)

    # Phase 1: Q' qb0 operands, in dc consumption order. Later phases are
    # emitted between compute sections (see bottom) so that evac/exp
    # instructions on the scalar queue aren't stuck behind a long run of
    # serialized DIRECT2D descriptor-generation slices.
    # ~512KB per DMA: each ring item pays ~1.5us of fixed issue+completion
    # latency, so smaller chunks cap per-ring throughput well below HBM BW.
    # first two d-chunks ride alone so the first matmul starts ~3us sooner
    xfers = []
    for sl in (slice(0, 1), slice(1, 2), slice(2, 4), slice(4, 6), slice(6, 8)):
        xfers.append((m32_s[:, sl, :], m32_r[:, sl, :]))
        xfers.append((qT_s[:, sl, :], qT_r[:, sl, :]))
    xfers.append((t2s_s, t2s))  # host ships t2s pre-transposed to [P, KVC]
    emit_xfers(xfers, [nc.sync, nc.scalar, nc.gpsimd])

    def phase2():  # scores operands (qT now fully loaded in phase 1)
        xfers = []
        for dc in range(0, DC, 2):
            xfers.append((kTb_s[:, dc : dc + 2, :], kTb_r[:, dc : dc + 2, :]))
        for dc in range(0, DC, 4):
            xfers.append((kT8_s[:, dc : dc + 4, :], kT8_r[:, dc : dc + 4, :]))
        emit_xfers(xfers, [nc.sync, nc.scalar, nc.gpsimd])

    def phase34():  # AX operand, then output projection operand
        xfers = []
        for c in range(0, KVC, 2):
            xfers.append((xv_s[:, c : c + 2, :], xv_r[:, c : c + 2, :]))
        xfers.append((bo2_s, bo2_bcast))
        for dc in range(0, DC, 2):
            xfers.append((n2_s[:, dc : dc + 2, :], n2_r[:, dc : dc + 2, :]))
        emit_xfers(xfers, [nc.sync, nc.gpsimd])

    # ---- Q'^T = M32.T @ qT ---------------------------------------------------
    qp = big.tile([P, DC, QL], CDT, tag="qp")  # Q'^T: [d'%128, d'//128, q]
    qp8 = None
    if KV8C < KVC:
        qp8 = big.tile([P, DC, QL], F8, tag="qp8", name="qp8")

    def qp_evac(ec, qb, ps):
        # one evac per psum bank, alternating engines, so the bank-release
        # drain is short; the fp8 copy for the DoubleRow scores is derived
        # from qp lazily (qp8_fill), off the bank-critical path.
        sl = slice(qb * N5, (qb + 1) * N5)
        if ec % 2 == 0:
            nc.scalar.activation(
                out=qp[:, ec, sl], in_=ps, func=AF.Identity, scale=1.0
            )
        else:
            nc.vector.tensor_copy(out=qp[:, ec, sl], in_=ps)

    def qp8_fill(qb):
        if qp8 is None:
            return
        sl = slice(qb * N5, (qb + 1) * N5)
        for ec in range(DC):
            nc.vector.tensor_copy(out=qp8[:, ec, sl], in_=qp[:, ec, sl])

    def qprime8(qb):
        pss = [
            qpool.tile([P, N5], F32, tag="qmm", name=f"qps{qb}_{ec}")
            for ec in range(DC)
        ]
        for dc in range(DC):
            for ec in range(DC):
                nc.tensor.matmul(
                    pss[ec],
                    lhsT=m32_s[:, dc, ec * P : (ec + 1) * P],
                    rhs=qT_s[:, dc, qb * N5 : (qb + 1) * N5],
                    start=(dc == 0),
                    stop=(dc == DC - 1),
                )
        for ec in range(DC):
            qp_evac(ec, qb, pss[ec])

    # ---- scores + exp + sums + AX, one kv pass per 512-query block ----------
    def scores_exp(qb):
        attnT = attn_pool.tile([P, KVC, N5], CDT, tag="attnT")
        qsl = slice(qb * N5, (qb + 1) * N5)
        for c in range(KVC):
            ps = psum.tile([P, N5], F32, tag="mm")
            if c < KV8C:  # bf16 path
                for dc in range(DC):
                    nc.tensor.matmul(
                        ps,
                        lhsT=kTb_s[:, dc, c * P : (c + 1) * P],
                        rhs=qp[:, dc, qsl],
                        start=(dc == 0),
                        stop=(dc == DC - 1),
                    )
            else:  # fp8 DoubleRow path: two d-chunks per instruction
                c8 = c - KV8C
                for dc in range(0, DC, 2):
                    nc.tensor.matmul(
                        ps,
                        lhsT=kT8_s[:, dc : dc + 2, c8 * P : (c8 + 1) * P],
                        rhs=qp8[:, dc : dc + 2, qsl],
                        start=(dc == 0),
                        stop=(dc == DC - 2),
                        perf_mode=DR,
                    )
            nc.scalar.activation(
                out=attnT[:, c, :],
                in_=ps,
                func=AF.Exp,
                scale=SCALE,
                bias=t2s_s[:, c : c + 1],
            )
        return attnT

    def sums_recip(attnT):
        ps_sum = psum_s.tile([P, N5 // P], F32, tag="sums")
        for s in range(N5 // P):
            for c in range(KVC):
                nc.tensor.matmul(
                    ps_sum[:, s : s + 1],
                    lhsT=attnT[:, c, s * P : (s + 1) * P],
                    rhs=ones[:, :1],
                    start=(c == 0),
                    stop=(c == KVC - 1),
                )
        r_s = evac.tile([P, N5 // P], F32, tag="recip")
        nc.vector.reciprocal(r_s, ps_sum)
        return r_s

    def ax_block(attnT):
        axT = attn_pool.tile([P, DC, N5], CDT, tag="axT")  # AX^T: [dv%128, m, q]
        for m in range(DC):
            ps = psum.tile([P, N5], F32, tag="mm")
            for c in range(KVC):
                nc.tensor.matmul(
                    ps,
                    lhsT=xv_s[:, c, m * P : (m + 1) * P],
                    rhs=attnT[:, c, :],
                    start=(c == 0),
                    stop=(c == KVC - 1),
                )
            nc.vector.tensor_copy(out=axT[:, m, :], in_=ps)
        return axT

    def out_block(qb, axT, r_s):
        for s in range(N5 // P):
            for nf in range(D // N5):
                ps = psum.tile([P, N5], F32, tag="mm")
                for m in range(DC):
                    nc.tensor.matmul(
                        ps,
                        lhsT=axT[:, m, s * P : (s + 1) * P],
                        rhs=n2_s[:, m, nf * N5 : (nf + 1) * N5],
                        start=(m == 0),
                        stop=(m == DC - 1),
                    )
                fin = evac.tile([P, N5], F32, tag="fin")
                nc.vector.scalar_tensor_tensor(
                    out=fin,
                    in0=ps,
                    scalar=r_s[:, s : s + 1],
                    in1=bo2_s[:, nf * N5 : (nf + 1) * N5],
                    op0=ALU.mult,
                    op1=ALU.add,
                )
                row0 = qb * N5 + s * P
                nc.sync.dma_start(
                    out=out[row0 : row0 + P, nf * N5 : (nf + 1) * N5], in_=fin
                )

    qprime8(0)
    phase2()
    qp8_fill(0)
    qprime8(1)
    phase34()
    qp8_fill(1)
    qpool_cm.__exit__(None, None, None)
    psum = ctx.enter_context(tc.tile_pool(name="psum", bufs=4, space="PSUM"))
    psum_s = ctx.enter_context(tc.tile_pool(name="psum_s", bufs=2, space="PSUM"))
    a0 = scores_exp(0)
    r0 = sums_recip(a0)
    x0 = ax_block(a0)
    out_block(0, x0, r0)
    a1 = scores_exp(1)
    r1 = sums_recip(a1)
    x1 = ax_block(a1)
    out_block(1, x1, r1)


def build_program():
    nc = bacc.Bacc(
        "TRN2", target_bir_lowering=False, debug=False, num_devices=NCORES
    )
    qT = nc.dram_tensor("qT", [D, QL], CDT, kind="ExternalInput").ap()
    kTb = nc.dram_tensor("kTb", [D, KVB], CDT, kind="ExternalInput").ap()
    kT8 = nc.dram_tensor("kT8", [D, SKV - KVB], F8, kind="ExternalInput").ap()
    xv = nc.dram_tensor("xv", [SKV, D], CDT, kind="ExternalInput").ap()
    m32 = nc.dram_tensor("m32", [D, D], CDT, kind="ExternalInput").ap()
    n2 = nc.dram_tensor("n2", [D, D], CDT, kind="ExternalInput").ap()
    t2s = nc.dram_tensor("t2s", [P, KVC], F32, kind="ExternalInput").ap()
    bo2 = nc.dram_tensor("bo2", [D], F32, kind="ExternalInput").ap()
    out = nc.dram_tensor("out", [QL, D], F32, kind="ExternalOutput").ap()

    with tile.TileContext(nc) as tc:
        with ExitStack() as ctx:
            _build_tile(ctx, tc, (qT, kTb, kT8, xv, m32, n2, t2s, bo2, out))
    nc.compile()
    return nc


def prep_in_maps(query, key, value, Wq, bq, Wk, bk, Wv, bv, Wo, bo):
    """Host-side shard prep: fold weights, slice, transpose, cast."""
    query = np.asarray(query, np.float32)
    key = np.asarray(key, np.float32)
    value = np.asarray(value, np.float32)
    Wq = np.asarray(Wq, np.float32)
    Wk = np.asarray(Wk, np.float32)
    Wv = np.asarray(Wv, np.float32)
    Wo = np.asarray(Wo, np.float32)
    bq = np.asarray(bq, np.float32)
    bv = np.asarray(bv, np.float32)
    bo = np.asarray(bo, np.float32)

    M32 = (Wq @ Wk.T) * MS
    N2 = Wv @ Wo
    ck = Wk @ bq  # per-kv score offset direction; zero when bq == 0
    shared = {
        "m32": M32.astype(NP_CDT),
        "n2": N2.astype(NP_CDT),
        "bo2": bv @ Wo + bo,
    }
    in_maps = []
    for b in range(B):
        kT = np.ascontiguousarray(key[b].T)
        kTbb = kT[:, :KVB].astype(NP_CDT)
        kT8b = kT[:, KVB:].astype(NP_F8)
        xvb = value[b].astype(NP_CDT)
        # pre-transposed to [P, KVC] so the DMA is 128 contiguous 64B runs
        # instead of 2048 four-byte runs (descriptor-generation cost)
        t2sb = np.ascontiguousarray(
            (SCALE * (key[b] @ ck)).astype(np.float32).reshape(KVC, P).T
        )
        for h in range(2):
            qTb = np.ascontiguousarray(query[b, h * QL : (h + 1) * QL].T).astype(
                NP_CDT
            )
            in_maps.append(
                {"qT": qTb, "kTb": kTbb, "kT8": kT8b, "xv": xvb, "t2s": t2sb,
                 **shared}
            )
    return in_maps


_NC_CACHE = None


def _get_nc():
    global _NC_CACHE
    if _NC_CACHE is None:
        _NC_CACHE = build_program()
    return _NC_CACHE


def run(inputs, **run_kwargs):
    nc = _get_nc()
    in_maps = prep_in_maps(**inputs)
    res = run_bass_kernel_spmd(nc, in_maps, core_ids=list(range(NCORES)), **run_kwargs)
    out = np.empty((B, SQ, D), np.float32)
    for b in range(B):
        for h in range(2):
            out[b, h * QL : (h + 1) * QL] = res.results[2 * b + h]["out"]
    return out, res


def kernel(query, key, value, Wq, bq, Wk, bk, Wv, bv, Wo, bo):
    out, _ = run(
        dict(
            query=query, key=key, value=value, Wq=Wq, bq=bq, Wk=Wk, bk=bk,
            Wv=Wv, bv=bv, Wo=Wo, bo=bo,
        )
    )
    return out


if __name__ == "__main__":
    rng = np.random.default_rng(0)
    ins = {
        "query": rng.standard_normal((B, SQ, D), dtype=np.float32),
        "key": rng.standard_normal((B, SKV, D), dtype=np.float32),
        "value": rng.standard_normal((B, SKV, D), dtype=np.float32),
        "Wq": (rng.standard_normal((D, D), dtype=np.float32) * 0.02),
        "bq": np.zeros(D, np.float32),
        "Wk": (rng.standard_normal((D, D), dtype=np.float32) * 0.02),
        "bk": np.zeros(D, np.float32),
        "Wv": (rng.standard_normal((D, D), dtype=np.float32) * 0.02),
        "bv": np.zeros(D, np.float32),
        "Wo": (rng.standard_normal((D, D), dtype=np.float32) * 0.02),
        "bo": np.zeros(D, np.float32),
    }
    out = kernel(**ins)
    print("kernel ran, out shape", out.shape)


# revision 37
# speedup vs baseline: 1.1061x; 1.0156x over previous
"""Trainium2 Bass kernel for single-head cross-attention (v3: folded weights).

Reference computation (B=4, Sq=Skv=2048, D=1024, fp32):
    Q = query @ Wq + bq ; K = key @ Wk + bk ; V = value @ Wv + bv
    out = softmax(Q K^T / sqrt(D)) V @ Wo + bo

Since no nonlinearity separates the projections from the score/output
matmuls, the host folds the weights (a static, per-model transform):
    M = Wq @ Wk.T        scores = (query @ M) @ key^T  (K proj eliminated)
    N = Wv @ Wo          out    = (attn @ value) @ N   (V proj eliminated)
Bias terms fold exactly: the per-kv offset key @ (Wk @ bq) becomes the
exp() activation bias; per-q offsets cancel against the softmax
denominator (we divide by the sums at the very end, so they never need
computing); bv @ Wo + bo is the output bias.

This removes 25% of the device FLOPs and, because each core can simply
be HANDED the full raw key/value for its batch, the K/V AllGathers of
v2 disappear entirely. Sharding: 8 shards = (batch b) x (query half h);
core 2*b+h computes output rows [h*1024,(h+1)*1024) of batch b. All
matmul operands are bf16 (fp8 DoubleRow was measured at 1.9e-2 rel err
against the 2e-2 budget - too close).

M is pre-scaled by 32 on the host so Q' = query @ 32M has entries O(15)
(fp32 PSUM doesn't care, but it keeps the bf16 store well-conditioned);
the exp scale absorbs the 2^-10.

Dataflow per core (all contractions land on SBUF partitions):
    Q'^T[e,q]  = M32.T @ qT        (lhsT=m32,  rhs=qT)
    S^T[kv,q]  = key @ Q'^T        (lhsT=kT,   rhs=Q'^T)
    A^T        = exp(S^T/1024 + t2s)            (unnormalized)
    AX^T[dv,q] = value.T @ A^T     (lhsT=xv,   rhs=A^T)
    sums[q,1]  = A @ ones          (lhsT=A^T,  rhs=ones)
    F[q,f]     = AX @ N            (lhsT=AX^T, rhs=n2)
    out        = F * (1/sums) + (bv @ Wo + bo)
"""

import sys

if "/opt/trn_rl_repo" not in sys.path:
    sys.path.insert(0, "/opt/trn_rl_repo")

from contextlib import ExitStack

import ml_dtypes
import numpy as np

import concourse.bass as bass
import concourse.mybir as mybir
import concourse.tile as tile
from concourse import bacc
from concourse.bass_utils import run_bass_kernel_spmd

B, SQ, SKV, D = 4, 2048, 2048, 1024
NCORES = 8
QL = SQ // 2  # local query rows per core
P = 128
DC = D // P  # feature chunks (8)
KVC = SKV // P  # kv chunks (16)
N5 = 512
NQB = QL // N5  # query blocks (2)
F32 = mybir.dt.float32
CDT = mybir.dt.bfloat16
F8 = mybir.dt.float8e4
NP_CDT = ml_dtypes.bfloat16
NP_F8 = ml_dtypes.float8_e4m3fn
MS = 32.0  # host pre-scale on M
SCALE = 1.0 / (32.0 * MS)  # exp scale: 1/sqrt(D) / MS

# kv chunks [KV8C:KVC) compute their scores in fp8e4 DoubleRow (2 k-rows per
# instruction, ~1.8x bf16 rate); softmax attenuates the quantization noise.
# Measured rel err 1.39e-2 at KV8C=8 vs the 2e-2 budget (all-bf16: 3.6e-3).
KV8C = 4  # first fp8 kv chunk; KVC to disable fp8 entirely
KVB = KV8C * P  # kv rows computed in bf16

AF = mybir.ActivationFunctionType
ALU = mybir.AluOpType
DR = mybir.MatmulPerfMode.DoubleRow


def _build_tile(ctx: ExitStack, tc, aps):
    nc = tc.nc
    qT, kTb, kT8, xv, m32, n2, t2s, bo2, out = aps

    big = ctx.enter_context(tc.tile_pool(name="big", bufs=1))
    attn_pool = ctx.enter_context(tc.tile_pool(name="attn", bufs=2))
    evac = ctx.enter_context(tc.tile_pool(name="evac", bufs=4))
    # Q' gets all 8 PSUM banks (one per ec) so each query block is a single
    # dc-outer sweep: the m32 walk then spans 13.9us of matmuls and stays
    # behind the DMA stream. The pool closes before psum/psum_s open.
    qpool_cm = tc.tile_pool(name="qpool", bufs=8, space="PSUM")
    qpool = qpool_cm.__enter__()

    # ---- input DMAs, spread across rings so they stream in parallel --------
    # Critical path: the first Q' psum group consumes every d-chunk of m32,
    # so m32 rides two rings (evens/odds) and qT's first query block leads
    # the third; kT (scores, needed ~15us in) gets the sync ring to itself.
    m32_r = m32.rearrange("(c p) e -> p c e", p=P)
    qT_r = qT.rearrange("(c p) q -> p c q", p=P)
    kTb_r = kTb.rearrange("(c p) n -> p c n", p=P)
    kT8_r = kT8.rearrange("(c p) n -> p c n", p=P)
    xv_r = xv.rearrange("(c p) n -> p c n", p=P)
    n2_r = n2.rearrange("(c p) f -> p c f", p=P)

    # Per-d-chunk DMAs: HWDGE descriptor-generation time is linear in the
    # number of contiguous runs, so one [:, :, slice] DMA costs the same
    # sequencer time as eight [:, c, :] DMAs but delays every consumer until
    # the whole thing is issued. The SDMA engines round-robin between rings
    # that have queued work at packet granularity, so EVERY ring must carry
    # earliest-deadline traffic first: stripe all tensors across the three
    # rings in global consumption order (m32/qT -> kT -> xv -> n2).
    m32_s = big.tile([P, DC, D], CDT, tag="m32")
    qT_s = big.tile([P, DC, QL], CDT, tag="qT")
    kTb_s = big.tile([P, DC, KVB], CDT, tag="kTb")
    kT8_s = big.tile([P, DC, SKV - KVB], F8, tag="kT8")
    xv_s = big.tile([P, KVC, D], CDT, tag="xv")
    n2_s = big.tile([P, DC, D], CDT, tag="n2")
    t2s_s = big.tile([P, KVC], F32, tag="t2s")
    bo2_s = big.tile([P, D], F32, tag="bo2")
    bo2_bcast = bass.AP(tensor=bo2.tensor, offset=bo2.offset, ap=[[0, P], bo2.ap[0]])
    ones = big.tile([P, 1], CDT, tag="ones")
    nc.vector.memset(ones, 1.0)

    # Phases 1-2 (before the exp stream occupies Scalar) use all three rings;
    # later phases avoid the scalar ring so DMA issues never delay the
    # exp/evac activations queued behind them.
    def emit_xfers(xfers, rings):
        for i, (dst, src) in enumerate(xfers):
            rings[i % len(rings)].dma_start(out=dst, in_=src)

    # Phase 1: Q' qb0 operands, in dc consumption order. Later phases are
    # emitted between compute sections (see bottom) so that evac/exp
    # instructions on the scalar queue aren't stuck behind a long run of
    # serialized DIRECT2D descriptor-generation slices.
    # ~512KB per DMA: each ring item pays ~1.5us of fixed issue+completion
    # latency, so smaller chunks cap per-ring throughput well below HBM BW.
    # first two d-chunks ride alone so the first matmul starts ~3us sooner
    xfers = []
    for sl in (slice(0, 1), slice(1, 2), slice(2, 4), slice(4, 6), slice(6, 8)):
        xfers.append((m32_s[:, sl, :], m32_r[:, sl, :]))
        xfers.append((qT_s[:, sl, :], qT_r[:, sl, :]))
    xfers.append((t2s_s, t2s))  # host ships t2s pre-transposed to [P, KVC]
    emit_xfers(xfers, [nc.sync, nc.scalar, nc.gpsimd])

    def phase2():  # scores operands (qT now fully loaded in phase 1)
        xfers = []
        for dc in range(0, DC, 2):
            xfers.append((kTb_s[:, dc : dc + 2, :], kTb_r[:, dc : dc + 2, :]))
        for dc in range(0, DC, 4):
            xfers.append((kT8_s[:, dc : dc + 4, :], kT8_r[:, dc : dc + 4, :]))
        emit_xfers(xfers, [nc.sync, nc.scalar, nc.gpsimd])

    def phase34():  # AX operand, then output projection operand
        xfers = []
        for c in range(0, KVC, 2):
            xfers.append((xv_s[:, c : c + 2, :], xv_r[:, c : c + 2, :]))
        xfers.append((bo2_s, bo2_bcast))
        for dc in range(0, DC, 2):
            xfers.append((n2_s[:, dc : dc + 2, :], n2_r[:, dc : dc + 2, :]))
        emit_xfers(xfers, [nc.sync, nc.gpsimd])

    # ---- Q'^T = M32.T @ qT ---------------------------------------------------
    qp = big.tile([P, DC, QL], CDT, tag="qp")  # Q'^T: [d'%128, d'//128, q]
    qp8 = None
    if KV8C < KVC:
        qp8 = big.tile([P, DC, QL], F8, tag="qp8", name="qp8")

    def qp_evac(ec, qb, ps):
        # one evac per psum bank, alternating engines, so the bank-release
        # drain is short; the fp8 copy for the DoubleRow scores is derived
        # from qp lazily (qp8_fill), off the bank-critical path.
        sl = slice(qb * N5, (qb + 1) * N5)
        if ec % 2 == 0:
            nc.scalar.activation(
                out=qp[:, ec, sl], in_=ps, func=AF.Identity, scale=1.0
            )
        else:
            nc.vector.tensor_copy(out=qp[:, ec, sl], in_=ps)

    def qp8_fill(qb):
        if qp8 is None:
            return
        sl = slice(qb * N5, (qb + 1) * N5)
        for ec in range(DC):
            nc.vector.tensor_copy(out=qp8[:, ec, sl], in_=qp[:, ec, sl])

    def qprime8(qb):
        pss = [
            qpool.tile([P, N5], F32, tag="qmm", name=f"qps{qb}_{ec}")
            for ec in range(DC)
        ]
        for dc in range(DC):
            for ec in range(DC):
                nc.tensor.matmul(
                    pss[ec],
                    lhsT=m32_s[:, dc, ec * P : (ec + 1) * P],
                    rhs=qT_s[:, dc, qb * N5 : (qb + 1) * N5],
                    start=(dc == 0),
                    stop=(dc == DC - 1),
                )
        for ec in range(DC):
            qp_evac(ec, qb, pss[ec])

    # ---- scores + exp + sums + AX, one kv pass per 512-query block ----------
    def scores_exp(qb):
        attnT = attn_pool.tile([P, KVC, N5], CDT, tag="attnT")
        qsl = slice(qb * N5, (qb + 1) * N5)
        for c in range(KVC):
            ps = psum.tile([P, N5], F32, tag="mm")
            if c < KV8C:  # bf16 path
                for dc in range(DC):
                    nc.tensor.matmul(
                        ps,
                        lhsT=kTb_s[:, dc, c * P : (c + 1) * P],
                        rhs=qp[:, dc, qsl],
                        start=(dc == 0),
                        stop=(dc == DC - 1),
                    )
            else:  # fp8 DoubleRow path: two d-chunks per instruction
                c8 = c - KV8C
                for dc in range(0, DC, 2):
                    nc.tensor.matmul(
                        ps,
                        lhsT=kT8_s[:, dc : dc + 2, c8 * P : (c8 + 1) * P],
                        rhs=qp8[:, dc : dc + 2, qsl],
                        start=(dc == 0),
                        stop=(dc == DC - 2),
                        perf_mode=DR,
                    )
            nc.scalar.activation(
                out=attnT[:, c, :],
                in_=ps,
                func=AF.Exp,
                scale=SCALE,
                bias=t2s_s[:, c : c + 1],
            )
        return attnT

    def sums_recip(attnT):
        ps_sum = psum_s.tile([P, N5 // P], F32, tag="sums")
        for s in range(N5 // P):
            for c in range(KVC):
                nc.tensor.matmul(
                    ps_sum[:, s : s + 1],
                    lhsT=attnT[:, c, s * P : (s + 1) * P],
                    rhs=ones[:, :1],
                    start=(c == 0),
                    stop=(c == KVC - 1),
                )
        r_s = evac.tile([P, N5 // P], F32, tag="recip")
        nc.vector.reciprocal(r_s, ps_sum)
        return r_s

    def ax_block(attnT):
        axT = attn_pool.tile([P, DC, N5], CDT, tag="axT")  # AX^T: [dv%128, m, q]
        for m in range(DC):
            ps = psum.tile([P, N5], F32, tag="mm")
            for c in range(KVC):
                nc.tensor.matmul(
                    ps,
                    lhsT=xv_s[:, c, m * P : (m + 1) * P],
                    rhs=attnT[:, c, :],
                    start=(c == 0),
                    stop=(c == KVC - 1),
                )
            nc.vector.tensor_copy(out=axT[:, m, :], in_=ps)
        return axT

    def out_block(qb, axT, r_s):
        for s in range(N5 // P):
            for nf in range(D // N5):
                ps = psum.tile([P, N5], F32, tag="mm")
                for m in range(DC):
                    nc.tensor.matmul(
                        ps,
                        lhsT=axT[:, m, s * P : (s + 1) * P],
                        rhs=n2_s[:, m, nf * N5 : (nf + 1) * N5],
                        start=(m == 0),
                        stop=(m == DC - 1),
                    )
                fin = evac.tile([P, N5], F32, tag="fin")
                nc.vector.scalar_tensor_tensor(
                    out=fin,
                    in0=ps,
                    scalar=r_s[:, s : s + 1],
                    in1=bo2_s[:, nf * N5 : (nf + 1) * N5],
                    op0=ALU.mult,
                    op1=ALU.add,
                )
                row0 = qb * N5 + s * P
                nc.sync.dma_start(
                    out=out[row0 : row0 + P, nf * N5 : (nf + 1) * N5], in_=fin
                )

    qprime8(0)
    phase2()
    qp8_fill(0)
    qprime8(1)
    phase34()
    qp8_fill(1)
    qpool_cm.__exit__(None, None, None)
    psum = ctx.enter_context(tc.tile_pool(name="psum", bufs=4, space="PSUM"))
    psum_s = ctx.enter_context(tc.tile_pool(name="psum_s", bufs=2, space="PSUM"))
    a0 = scores_exp(0)
    r0 = sums_recip(a0)
    x0 = ax_block(a0)
    out_block(0, x0, r0)
    a1 = scores_exp(1)
    r1 = sums_recip(a1)
    x1 = ax_block(a1)
    out_block(1, x1, r1)


def build_program():
    nc = bacc.Bacc(
        "TRN2", target_bir_lowering=False, debug=False, num_devices=NCORES
    )
    qT = nc.dram_tensor("qT", [D, QL], CDT, kind="ExternalInput").ap()
    kTb = nc.dram_tensor("kTb", [D, KVB], CDT, kind="ExternalInput").ap()
    kT8 = nc.dram_tensor("kT8", [D, SKV - KVB], F8, kind="ExternalInput").ap()
    xv = nc.dram_tensor("xv", [SKV, D], CDT, kind="ExternalInput").ap()
    m32 = nc.dram_tensor("m32", [D, D], CDT, kind="ExternalInput").ap()
    n2 = nc.dram_tensor("n2", [D, D], CDT, kind="ExternalInput").ap()
    t2s = nc.dram_tensor("t2s", [P, KVC], F32, kind="ExternalInput").ap()
    bo2 = nc.dram_tensor("bo2", [D], F32, kind="ExternalInput").ap()
    out = nc.dram_tensor("out", [QL, D], F32, kind="ExternalOutput").ap()

    with tile.TileContext(nc) as tc:
        with ExitStack() as ctx:
            _build_tile(ctx, tc, (qT, kTb, kT8, xv, m32, n2, t2s, bo2, out))
    nc.compile()
    return nc


def prep_in_maps(query, key, value, Wq, bq, Wk, bk, Wv, bv, Wo, bo):
    """Host-side shard prep: fold weights, slice, transpose, cast."""
    query = np.asarray(query, np.float32)
    key = np.asarray(key, np.float32)
    value = np.asarray(value, np.float32)
    Wq = np.asarray(Wq, np.float32)
    Wk = np.asarray(Wk, np.float32)
    Wv = np.asarray(Wv, np.float32)
    Wo = np.asarray(Wo, np.float32)
    bq = np.asarray(bq, np.float32)
    bv = np.asarray(bv, np.float32)
    bo = np.asarray(bo, np.float32)

    M32 = (Wq @ Wk.T) * MS
    N2 = Wv @ Wo
    ck = Wk @ bq  # per-kv score offset direction; zero when bq == 0
    shared = {
        "m32": M32.astype(NP_CDT),
        "n2": N2.astype(NP_CDT),
        "bo2": bv @ Wo + bo,
    }
    in_maps = []
    for b in range(B):
        kT = np.ascontiguousarray(key[b].T)
        kTbb = kT[:, :KVB].astype(NP_CDT)
        kT8b = kT[:, KVB:].astype(NP_F8)
        xvb = value[b].astype(NP_CDT)
        # pre-transposed to [P, KVC] so the DMA is 128 contiguous 64B runs
        # instead of 2048 four-byte runs (descriptor-generation cost)
        t2sb = np.ascontiguousarray(
            (SCALE * (key[b] @ ck)).astype(np.float32).reshape(KVC, P).T
        )
        for h in range(2):
            qTb = np.ascontiguousarray(query[b, h * QL : (h + 1) * QL].T).astype(
                NP_CDT
            )
            in_maps.append(
                {"qT": qTb, "kTb": kTbb, "kT8": kT8b, "xv": xvb, "t2s": t2sb,
                 **shared}
            )
    return in_maps


_NC_CACHE = None


def _get_nc():
    global _NC_CACHE
    if _NC_CACHE is None:
        _NC_CACHE = build_program()
    return _NC_CACHE


def run(inputs, **run_kwargs):
    nc = _get_nc()
    in_maps = prep_in_maps(**inputs)
    res = run_bass_kernel_spmd(nc, in_maps, core_ids=list(range(NCORES)), **run_kwargs)
    out = np.empty((B, SQ, D), np.float32)
    for b in range(B):
        for h in range(2):
            out[b, h * QL : (h + 1) * QL] = res.results[2 * b + h]["out"]
    return out, res


def kernel(query, key, value, Wq, bq, Wk, bk, Wv, bv, Wo, bo):
    out, _ = run(
        dict(
            query=query, key=key, value=value, Wq=Wq, bq=bq, Wk=Wk, bk=bk,
            Wv=Wv, bv=bv, Wo=Wo, bo=bo,
        )
    )
    return out


if __name__ == "__main__":
    rng = np.random.default_rng(0)
    ins = {
        "query": rng.standard_normal((B, SQ, D), dtype=np.float32),
        "key": rng.standard_normal((B, SKV, D), dtype=np.float32),
        "value": rng.standard_normal((B, SKV, D), dtype=np.float32),
        "Wq": (rng.standard_normal((D, D), dtype=np.float32) * 0.02),
        "bq": np.zeros(D, np.float32),
        "Wk": (rng.standard_normal((D, D), dtype=np.float32) * 0.02),
        "bk": np.zeros(D, np.float32),
        "Wv": (rng.standard_normal((D, D), dtype=np.float32) * 0.02),
        "bv": np.zeros(D, np.float32),
        "Wo": (rng.standard_normal((D, D), dtype=np.float32) * 0.02),
        "bo": np.zeros(D, np.float32),
    }
    out = kernel(**ins)
    print("kernel ran, out shape", out.shape)


# revision 38
# speedup vs baseline: 1.1081x; 1.0018x over previous
"""Trainium2 Bass kernel for single-head cross-attention (v3: folded weights).

Reference computation (B=4, Sq=Skv=2048, D=1024, fp32):
    Q = query @ Wq + bq ; K = key @ Wk + bk ; V = value @ Wv + bv
    out = softmax(Q K^T / sqrt(D)) V @ Wo + bo

Since no nonlinearity separates the projections from the score/output
matmuls, the host folds the weights (a static, per-model transform):
    M = Wq @ Wk.T        scores = (query @ M) @ key^T  (K proj eliminated)
    N = Wv @ Wo          out    = (attn @ value) @ N   (V proj eliminated)
Bias terms fold exactly: the per-kv offset key @ (Wk @ bq) becomes the
exp() activation bias; per-q offsets cancel against the softmax
denominator (we divide by the sums at the very end, so they never need
computing); bv @ Wo + bo is the output bias.

This removes 25% of the device FLOPs and, because each core can simply
be HANDED the full raw key/value for its batch, the K/V AllGathers of
v2 disappear entirely. Sharding: 8 shards = (batch b) x (query half h);
core 2*b+h computes output rows [h*1024,(h+1)*1024) of batch b. All
matmul operands are bf16 (fp8 DoubleRow was measured at 1.9e-2 rel err
against the 2e-2 budget - too close).

M is pre-scaled by 32 on the host so Q' = query @ 32M has entries O(15)
(fp32 PSUM doesn't care, but it keeps the bf16 store well-conditioned);
the exp scale absorbs the 2^-10.

Dataflow per core (all contractions land on SBUF partitions):
    Q'^T[e,q]  = M32.T @ qT        (lhsT=m32,  rhs=qT)
    S^T[kv,q]  = key @ Q'^T        (lhsT=kT,   rhs=Q'^T)
    A^T        = exp(S^T/1024 + t2s)            (unnormalized)
    AX^T[dv,q] = value.T @ A^T     (lhsT=xv,   rhs=A^T)
    sums[q,1]  = A @ ones          (lhsT=A^T,  rhs=ones)
    F[q,f]     = AX @ N            (lhsT=AX^T, rhs=n2)
    out        = F * (1/sums) + (bv @ Wo + bo)
"""

import sys

if "/opt/trn_rl_repo" not in sys.path:
    sys.path.insert(0, "/opt/trn_rl_repo")

from contextlib import ExitStack

import ml_dtypes
import numpy as np

import concourse.bass as bass
import concourse.mybir as mybir
import concourse.tile as tile
from concourse import bacc
from concourse.bass_utils import run_bass_kernel_spmd

B, SQ, SKV, D = 4, 2048, 2048, 1024
NCORES = 8
QL = SQ // 2  # local query rows per core
P = 128
DC = D // P  # feature chunks (8)
KVC = SKV // P  # kv chunks (16)
N5 = 512
NQB = QL // N5  # query blocks (2)
F32 = mybir.dt.float32
CDT = mybir.dt.bfloat16
F8 = mybir.dt.float8e4
NP_CDT = ml_dtypes.bfloat16
NP_F8 = ml_dtypes.float8_e4m3fn
MS = 32.0  # host pre-scale on M
SCALE = 1.0 / (32.0 * MS)  # exp scale: 1/sqrt(D) / MS

# kv chunks [KV8C:KVC) compute their scores in fp8e4 DoubleRow (2 k-rows per
# instruction, ~1.8x bf16 rate); softmax attenuates the quantization noise.
# Measured rel err 1.39e-2 at KV8C=8 vs the 2e-2 budget (all-bf16: 3.6e-3).
KV8C = 4  # first fp8 kv chunk; KVC to disable fp8 entirely
KVB = KV8C * P  # kv rows computed in bf16

AF = mybir.ActivationFunctionType
ALU = mybir.AluOpType
DR = mybir.MatmulPerfMode.DoubleRow


def _build_tile(ctx: ExitStack, tc, aps):
    nc = tc.nc
    qT, kTb, kT8, xv, m32, n2, t2s, bo2, out = aps

    big = ctx.enter_context(tc.tile_pool(name="big", bufs=1))
    attn_pool = ctx.enter_context(tc.tile_pool(name="attn", bufs=2))
    evac = ctx.enter_context(tc.tile_pool(name="evac", bufs=4))
    # Q' gets all 8 PSUM banks (one per ec) so each query block is a single
    # dc-outer sweep: the m32 walk then spans 13.9us of matmuls and stays
    # behind the DMA stream. The pool closes before psum/psum_s open.
    qpool_cm = tc.tile_pool(name="qpool", bufs=8, space="PSUM")
    qpool = qpool_cm.__enter__()

    # ---- input DMAs, spread across rings so they stream in parallel --------
    # Critical path: the first Q' psum group consumes every d-chunk of m32,
    # so m32 rides two rings (evens/odds) and qT's first query block leads
    # the third; kT (scores, needed ~15us in) gets the sync ring to itself.
    m32_r = m32.rearrange("(c p) e -> p c e", p=P)
    qT_r = qT.rearrange("(c p) q -> p c q", p=P)
    kTb_r = kTb.rearrange("(c p) n -> p c n", p=P)
    kT8_r = kT8.rearrange("(c p) n -> p c n", p=P)
    xv_r = xv.rearrange("(c p) n -> p c n", p=P)
    n2_r = n2.rearrange("(c p) f -> p c f", p=P)

    # Per-d-chunk DMAs: HWDGE descriptor-generation time is linear in the
    # number of contiguous runs, so one [:, :, slice] DMA costs the same
    # sequencer time as eight [:, c, :] DMAs but delays every consumer until
    # the whole thing is issued. The SDMA engines round-robin between rings
    # that have queued work at packet granularity, so EVERY ring must carry
    # earliest-deadline traffic first: stripe all tensors across the three
    # rings in global consumption order (m32/qT -> kT -> xv -> n2).
    m32_s = big.tile([P, DC, D], CDT, tag="m32")
    qT_s = big.tile([P, DC, QL], CDT, tag="qT")
    kTb_s = big.tile([P, DC, KVB], CDT, tag="kTb")
    kT8_s = big.tile([P, DC, SKV - KVB], F8, tag="kT8")
    xv_s = big.tile([P, KVC, D], CDT, tag="xv")
    n2_s = big.tile([P, DC, D], CDT, tag="n2")
    t2s_s = big.tile([P, KVC], F32, tag="t2s")
    bo2_s = big.tile([P, D], F32, tag="bo2")
    bo2_bcast = bass.AP(tensor=bo2.tensor, offset=bo2.offset, ap=[[0, P], bo2.ap[0]])
    ones = big.tile([P, 1], CDT, tag="ones")
    nc.vector.memset(ones, 1.0)

    # Phases 1-2 (before the exp stream occupies Scalar) use all three rings;
    # later phases avoid the scalar ring so DMA issues never delay the
    # exp/evac activations queued behind them.
    def emit_xfers(xfers, rings):
        for i, (dst, src) in enumerate(xfers):
            rings[i % len(rings)].dma_start(out=dst, in_=src)

    # Phase 1: Q' qb0 operands, in dc consumption order. Later phases are
    # emitted between compute sections (see bottom) so that evac/exp
    # instructions on the scalar queue aren't stuck behind a long run of
    # serialized DIRECT2D descriptor-generation slices.
    # ~512KB per DMA: each ring item pays ~1.5us of fixed issue+completion
    # latency, so smaller chunks cap per-ring throughput well below HBM BW.
    # single d-chunks up front so the first matmuls start sooner and the
    # dc-loop never outruns delivery; pairs later to amortize per-DMA cost
    xfers = []
    for sl in (
        slice(0, 1), slice(1, 2), slice(2, 3), slice(3, 4),
        slice(4, 6), slice(6, 8),
    ):
        xfers.append((m32_s[:, sl, :], m32_r[:, sl, :]))
        xfers.append((qT_s[:, sl, :], qT_r[:, sl, :]))
    xfers.append((t2s_s, t2s))  # host ships t2s pre-transposed to [P, KVC]
    emit_xfers(xfers, [nc.sync, nc.scalar, nc.gpsimd])

    def phase2():  # scores operands (qT now fully loaded in phase 1)
        xfers = []
        for dc in range(0, DC, 2):
            xfers.append((kTb_s[:, dc : dc + 2, :], kTb_r[:, dc : dc + 2, :]))
        for dc in range(0, DC, 4):
            xfers.append((kT8_s[:, dc : dc + 4, :], kT8_r[:, dc : dc + 4, :]))
        emit_xfers(xfers, [nc.sync, nc.scalar, nc.gpsimd])

    def phase34():  # AX operand, then output projection operand
        xfers = []
        for c in range(0, KVC, 2):
            xfers.append((xv_s[:, c : c + 2, :], xv_r[:, c : c + 2, :]))
        xfers.append((bo2_s, bo2_bcast))
        for dc in range(0, DC, 2):
            xfers.append((n2_s[:, dc : dc + 2, :], n2_r[:, dc : dc + 2, :]))
        emit_xfers(xfers, [nc.sync, nc.gpsimd])

    # ---- Q'^T = M32.T @ qT ---------------------------------------------------
    qp = big.tile([P, DC, QL], CDT, tag="qp")  # Q'^T: [d'%128, d'//128, q]
    qp8 = None
    if KV8C < KVC:
        qp8 = big.tile([P, DC, QL], F8, tag="qp8", name="qp8")

    def qp_evac(ec, qb, ps):
        # one evac per psum bank, alternating engines, so the bank-release
        # drain is short; the fp8 copy for the DoubleRow scores is derived
        # from qp lazily (qp8_fill), off the bank-critical path.
        sl = slice(qb * N5, (qb + 1) * N5)
        if ec % 2 == 0:
            nc.scalar.activation(
                out=qp[:, ec, sl], in_=ps, func=AF.Identity, scale=1.0
            )
        else:
            nc.vector.tensor_copy(out=qp[:, ec, sl], in_=ps)

    def qp8_fill(qb):
        if qp8 is None:
            return
        sl = slice(qb * N5, (qb + 1) * N5)
        for ec in range(DC):
            nc.vector.tensor_copy(out=qp8[:, ec, sl], in_=qp[:, ec, sl])

    def qprime8(qb):
        pss = [
            qpool.tile([P, N5], F32, tag="qmm", name=f"qps{qb}_{ec}")
            for ec in range(DC)
        ]
        for dc in range(DC):
            for ec in range(DC):
                nc.tensor.matmul(
                    pss[ec],
                    lhsT=m32_s[:, dc, ec * P : (ec + 1) * P],
                    rhs=qT_s[:, dc, qb * N5 : (qb + 1) * N5],
                    start=(dc == 0),
                    stop=(dc == DC - 1),
                )
        for ec in range(DC):
            qp_evac(ec, qb, pss[ec])

    # ---- scores + exp + sums + AX, one kv pass per 512-query block ----------
    def scores_exp(qb):
        attnT = attn_pool.tile([P, KVC, N5], CDT, tag="attnT")
        qsl = slice(qb * N5, (qb + 1) * N5)
        for c in range(KVC):
            ps = psum.tile([P, N5], F32, tag="mm")
            if c < KV8C:  # bf16 path
                for dc in range(DC):
                    nc.tensor.matmul(
                        ps,
                        lhsT=kTb_s[:, dc, c * P : (c + 1) * P],
                        rhs=qp[:, dc, qsl],
                        start=(dc == 0),
                        stop=(dc == DC - 1),
                    )
            else:  # fp8 DoubleRow path: two d-chunks per instruction
                c8 = c - KV8C
                for dc in range(0, DC, 2):
                    nc.tensor.matmul(
                        ps,
                        lhsT=kT8_s[:, dc : dc + 2, c8 * P : (c8 + 1) * P],
                        rhs=qp8[:, dc : dc + 2, qsl],
                        start=(dc == 0),
                        stop=(dc == DC - 2),
                        perf_mode=DR,
                    )
            nc.scalar.activation(
                out=attnT[:, c, :],
                in_=ps,
                func=AF.Exp,
                scale=SCALE,
                bias=t2s_s[:, c : c + 1],
            )
        return attnT

    def sums_recip(attnT):
        ps_sum = psum_s.tile([P, N5 // P], F32, tag="sums")
        for s in range(N5 // P):
            for c in range(KVC):
                nc.tensor.matmul(
                    ps_sum[:, s : s + 1],
                    lhsT=attnT[:, c, s * P : (s + 1) * P],
                    rhs=ones[:, :1],
                    start=(c == 0),
                    stop=(c == KVC - 1),
                )
        r_s = evac.tile([P, N5 // P], F32, tag="recip")
        nc.vector.reciprocal(r_s, ps_sum)
        return r_s

    def ax_block(attnT):
        axT = attn_pool.tile([P, DC, N5], CDT, tag="axT")  # AX^T: [dv%128, m, q]
        for m in range(DC):
            ps = psum.tile([P, N5], F32, tag="mm")
            for c in range(KVC):
                nc.tensor.matmul(
                    ps,
                    lhsT=xv_s[:, c, m * P : (m + 1) * P],
                    rhs=attnT[:, c, :],
                    start=(c == 0),
                    stop=(c == KVC - 1),
                )
            nc.vector.tensor_copy(out=axT[:, m, :], in_=ps)
        return axT

    def out_block(qb, axT, r_s):
        for s in range(N5 // P):
            for nf in range(D // N5):
                ps = psum.tile([P, N5], F32, tag="mm")
                for m in range(DC):
                    nc.tensor.matmul(
                        ps,
                        lhsT=axT[:, m, s * P : (s + 1) * P],
                        rhs=n2_s[:, m, nf * N5 : (nf + 1) * N5],
                        start=(m == 0),
                        stop=(m == DC - 1),
                    )
                fin = evac.tile([P, N5], F32, tag="fin")
                nc.vector.scalar_tensor_tensor(
                    out=fin,
                    in0=ps,
                    scalar=r_s[:, s : s + 1],
                    in1=bo2_s[:, nf * N5 : (nf + 1) * N5],
                    op0=ALU.mult,
                    op1=ALU.add,
                )
                row0 = qb * N5 + s * P
                nc.sync.dma_start(
                    out=out[row0 : row0 + P, nf * N5 : (nf + 1) * N5], in_=fin
                )

    qprime8(0)
    phase2()
    qp8_fill(0)
    qprime8(1)
    phase34()
    qp8_fill(1)
    qpool_cm.__exit__(None, None, None)
    psum = ctx.enter_context(tc.tile_pool(name="psum", bufs=4, space="PSUM"))
    psum_s = ctx.enter_context(tc.tile_pool(name="psum_s", bufs=2, space="PSUM"))
    a0 = scores_exp(0)
    r0 = sums_recip(a0)
    x0 = ax_block(a0)
    out_block(0, x0, r0)
    a1 = scores_exp(1)
    r1 = sums_recip(a1)
    x1 = ax_block(a1)
    out_block(1, x1, r1)


def build_program():
    nc = bacc.Bacc(
        "TRN2", target_bir_lowering=False, debug=False, num_devices=NCORES
    )
    qT = nc.dram_tensor("qT", [D, QL], CDT, kind="ExternalInput").ap()
    kTb = nc.dram_tensor("kTb", [D, KVB], CDT, kind="ExternalInput").ap()
    kT8 = nc.dram_tensor("kT8", [D, SKV - KVB], F8, kind="ExternalInput").ap()
    xv = nc.dram_tensor("xv", [SKV, D], CDT, kind="ExternalInput").ap()
    m32 = nc.dram_tensor("m32", [D, D], CDT, kind="ExternalInput").ap()
    n2 = nc.dram_tensor("n2", [D, D], CDT, kind="ExternalInput").ap()
    t2s = nc.dram_tensor("t2s", [P, KVC], F32, kind="ExternalInput").ap()
    bo2 = nc.dram_tensor("bo2", [D], F32, kind="ExternalInput").ap()
    out = nc.dram_tensor("out", [QL, D], F32, kind="ExternalOutput").ap()

    with tile.TileContext(nc) as tc:
        with ExitStack() as ctx:
            _build_tile(ctx, tc, (qT, kTb, kT8, xv, m32, n2, t2s, bo2, out))
    nc.compile()
    return nc


def prep_in_maps(query, key, value, Wq, bq, Wk, bk, Wv, bv, Wo, bo):
    """Host-side shard prep: fold weights, slice, transpose, cast."""
    query = np.asarray(query, np.float32)
    key = np.asarray(key, np.float32)
    value = np.asarray(value, np.float32)
    Wq = np.asarray(Wq, np.float32)
    Wk = np.asarray(Wk, np.float32)
    Wv = np.asarray(Wv, np.float32)
    Wo = np.asarray(Wo, np.float32)
    bq = np.asarray(bq, np.float32)
    bv = np.asarray(bv, np.float32)
    bo = np.asarray(bo, np.float32)

    M32 = (Wq @ Wk.T) * MS
    N2 = Wv @ Wo
    ck = Wk @ bq  # per-kv score offset direction; zero when bq == 0
    shared = {
        "m32": M32.astype(NP_CDT),
        "n2": N2.astype(NP_CDT),
        "bo2": bv @ Wo + bo,
    }
    in_maps = []
    for b in range(B):
        kT = np.ascontiguousarray(key[b].T)
        kTbb = kT[:, :KVB].astype(NP_CDT)
        kT8b = kT[:, KVB:].astype(NP_F8)
        xvb = value[b].astype(NP_CDT)
        # pre-transposed to [P, KVC] so the DMA is 128 contiguous 64B runs
        # instead of 2048 four-byte runs (descriptor-generation cost)
        t2sb = np.ascontiguousarray(
            (SCALE * (key[b] @ ck)).astype(np.float32).reshape(KVC, P).T
        )
        for h in range(2):
            qTb = np.ascontiguousarray(query[b, h * QL : (h + 1) * QL].T).astype(
                NP_CDT
            )
            in_maps.append(
                {"qT": qTb, "kTb": kTbb, "kT8": kT8b, "xv": xvb, "t2s": t2sb,
                 **shared}
            )
    return in_maps


_NC_CACHE = None


def _get_nc():
    global _NC_CACHE
    if _NC_CACHE is None:
        _NC_CACHE = build_program()
    return _NC_CACHE


def run(inputs, **run_kwargs):
    nc = _get_nc()
    in_maps = prep_in_maps(**inputs)
    res = run_bass_kernel_spmd(nc, in_maps, core_ids=list(range(NCORES)), **run_kwargs)
    out = np.empty((B, SQ, D), np.float32)
    for b in range(B):
        for h in range(2):
            out[b, h * QL : (h + 1) * QL] = res.results[2 * b + h]["out"]
    return out, res


def kernel(query, key, value, Wq, bq, Wk, bk, Wv, bv, Wo, bo):
    out, _ = run(
        dict(
            query=query, key=key, value=value, Wq=Wq, bq=bq, Wk=Wk, bk=bk,
            Wv=Wv, bv=bv, Wo=Wo, bo=bo,
        )
    )
    return out


if __name__ == "__main__":
    rng = np.random.default_rng(0)
    ins = {
        "query": rng.standard_normal((B, SQ, D), dtype=np.float32),
        "key": rng.standard_normal((B, SKV, D), dtype=np.float32),
        "value": rng.standard_normal((B, SKV, D), dtype=np.float32),
        "Wq": (rng.standard_normal((D, D), dtype=np.float32) * 0.02),
        "bq": np.zeros(D, np.float32),
        "Wk": (rng.standard_normal((D, D), dtype=np.float32) * 0.02),
        "bk": np.zeros(D, np.float32),
        "Wv": (rng.standard_normal((D, D), dtype=np.float32) * 0.02),
        "bv": np.zeros(D, np.float32),
        "Wo": (rng.standard_normal((D, D), dtype=np.float32) * 0.02),
        "bo": np.zeros(D, np.float32),
    }
    out = kernel(**ins)
    print("kernel ran, out shape", out.shape)


# revision 39
# speedup vs baseline: 1.1353x; 1.0246x over previous
"""Trainium2 Bass kernel for single-head cross-attention (v3: folded weights).

Reference computation (B=4, Sq=Skv=2048, D=1024, fp32):
    Q = query @ Wq + bq ; K = key @ Wk + bk ; V = value @ Wv + bv
    out = softmax(Q K^T / sqrt(D)) V @ Wo + bo

Since no nonlinearity separates the projections from the score/output
matmuls, the host folds the weights (a static, per-model transform):
    M = Wq @ Wk.T        scores = (query @ M) @ key^T  (K proj eliminated)
    N = Wv @ Wo          out    = (attn @ value) @ N   (V proj eliminated)
Bias terms fold exactly: the per-kv offset key @ (Wk @ bq) becomes the
exp() activation bias; per-q offsets cancel against the softmax
denominator (we divide by the sums at the very end, so they never need
computing); bv @ Wo + bo is the output bias.

This removes 25% of the device FLOPs and, because each core can simply
be HANDED the full raw key/value for its batch, the K/V AllGathers of
v2 disappear entirely. Sharding: 8 shards = (batch b) x (query half h);
core 2*b+h computes output rows [h*1024,(h+1)*1024) of batch b. All
matmul operands are bf16 (fp8 DoubleRow was measured at 1.9e-2 rel err
against the 2e-2 budget - too close).

M is pre-scaled by 32 on the host so Q' = query @ 32M has entries O(15)
(fp32 PSUM doesn't care, but it keeps the bf16 store well-conditioned);
the exp scale absorbs the 2^-10.

Dataflow per core (all contractions land on SBUF partitions):
    Q'^T[e,q]  = M32.T @ qT        (lhsT=m32,  rhs=qT)
    S^T[kv,q]  = key @ Q'^T        (lhsT=kT,   rhs=Q'^T)
    A^T        = exp(S^T/1024 + t2s)            (unnormalized)
    AX^T[dv,q] = value.T @ A^T     (lhsT=xv,   rhs=A^T)
    sums[q,1]  = A @ ones          (lhsT=A^T,  rhs=ones)
    F[q,f]     = AX @ N            (lhsT=AX^T, rhs=n2)
    out        = F * (1/sums) + (bv @ Wo + bo)
"""

import sys

if "/opt/trn_rl_repo" not in sys.path:
    sys.path.insert(0, "/opt/trn_rl_repo")

from contextlib import ExitStack

import ml_dtypes
import numpy as np

import concourse.bass as bass
import concourse.mybir as mybir
import concourse.tile as tile
from concourse import bacc
from concourse.bass_utils import run_bass_kernel_spmd

B, SQ, SKV, D = 4, 2048, 2048, 1024
NCORES = 8
QL = SQ // 2  # local query rows per core
P = 128
DC = D // P  # feature chunks (8)
KVC = SKV // P  # kv chunks (16)
N5 = 512
NQB = QL // N5  # query blocks (2)
F32 = mybir.dt.float32
CDT = mybir.dt.bfloat16
F8 = mybir.dt.float8e4
NP_CDT = ml_dtypes.bfloat16
NP_F8 = ml_dtypes.float8_e4m3fn
MS = 32.0  # host pre-scale on M
SCALE = 1.0 / (32.0 * MS)  # exp scale: 1/sqrt(D) / MS

# kv chunks [KV8C:KVC) compute their scores in fp8e4 DoubleRow (2 k-rows per
# instruction, ~1.8x bf16 rate); softmax attenuates the quantization noise.
# Measured rel err 1.39e-2 at KV8C=8 vs the 2e-2 budget (all-bf16: 3.6e-3).
KV8C = 4  # first fp8 kv chunk; KVC to disable fp8 entirely
KVB = KV8C * P  # kv rows computed in bf16

AF = mybir.ActivationFunctionType
ALU = mybir.AluOpType
DR = mybir.MatmulPerfMode.DoubleRow


def _build_tile(ctx: ExitStack, tc, aps):
    nc = tc.nc
    qT, kTb, kT8, xv, m32, n2, t2s, bo2, out = aps

    big = ctx.enter_context(tc.tile_pool(name="big", bufs=1))
    attn_pool = ctx.enter_context(tc.tile_pool(name="attn", bufs=2))
    evac = ctx.enter_context(tc.tile_pool(name="evac", bufs=4))
    # Q' gets all 8 PSUM banks (one per ec) so each query block is a single
    # dc-outer sweep: the m32 walk then spans 13.9us of matmuls and stays
    # behind the DMA stream. The pool closes before psum/psum_s open.
    qpool_cm = tc.tile_pool(name="qpool", bufs=8, space="PSUM")
    qpool = qpool_cm.__enter__()

    # ---- input DMAs, spread across rings so they stream in parallel --------
    # Critical path: the first Q' psum group consumes every d-chunk of m32,
    # so m32 rides two rings (evens/odds) and qT's first query block leads
    # the third; kT (scores, needed ~15us in) gets the sync ring to itself.
    m32_r = m32.rearrange("(c p) e -> p c e", p=P)
    qT_r = qT.rearrange("(c p) q -> p c q", p=P)
    kTb_r = kTb.rearrange("(c p) n -> p c n", p=P)
    kT8_r = kT8.rearrange("(c p) n -> p c n", p=P)
    xv_r = xv.rearrange("(c p) n -> p c n", p=P)
    n2_r = n2.rearrange("(c p) f -> p c f", p=P)

    # Per-d-chunk DMAs: HWDGE descriptor-generation time is linear in the
    # number of contiguous runs, so one [:, :, slice] DMA costs the same
    # sequencer time as eight [:, c, :] DMAs but delays every consumer until
    # the whole thing is issued. The SDMA engines round-robin between rings
    # that have queued work at packet granularity, so EVERY ring must carry
    # earliest-deadline traffic first: stripe all tensors across the three
    # rings in global consumption order (m32/qT -> kT -> xv -> n2).
    m32_s = big.tile([P, DC, D], CDT, tag="m32")
    qT_s = big.tile([P, DC, QL], CDT, tag="qT")
    kTb_s = big.tile([P, DC, KVB], CDT, tag="kTb")
    kT8_s = big.tile([P, DC, SKV - KVB], F8, tag="kT8")
    xv_s = big.tile([P, KVC, D], CDT, tag="xv")
    n2_s = big.tile([P, DC, D], CDT, tag="n2")
    t2s_s = big.tile([P, KVC], F32, tag="t2s")
    bo2_s = big.tile([P, D], F32, tag="bo2")
    bo2_bcast = bass.AP(tensor=bo2.tensor, offset=bo2.offset, ap=[[0, P], bo2.ap[0]])
    ones = big.tile([P, 1], CDT, tag="ones")
    nc.vector.memset(ones, 1.0)

    # Phases 1-2 (before the exp stream occupies Scalar) use all three rings;
    # later phases avoid the scalar ring so DMA issues never delay the
    # exp/evac activations queued behind them.
    def emit_xfers(xfers, rings):
        for i, (dst, src) in enumerate(xfers):
            rings[i % len(rings)].dma_start(out=dst, in_=src)

    # Phase 1: Q' qb0 operands, in dc consumption order. Later phases are
    # emitted between compute sections (see bottom) so that evac/exp
    # instructions on the scalar queue aren't stuck behind a long run of
    # serialized DIRECT2D descriptor-generation slices.
    # ~512KB per DMA: each ring item pays ~1.5us of fixed issue+completion
    # latency, so smaller chunks cap per-ring throughput well below HBM BW.
    # single d-chunks up front so the first matmuls start sooner and the
    # dc-loop never outruns delivery; pairs later to amortize per-DMA cost
    xfers = []
    for dc in range(DC):
        xfers.append((m32_s[:, dc, :], m32_r[:, dc, :]))
        xfers.append((qT_s[:, dc, :], qT_r[:, dc, :]))
    xfers.append((t2s_s, t2s))  # host ships t2s pre-transposed to [P, KVC]
    emit_xfers(xfers, [nc.sync, nc.scalar, nc.gpsimd])

    def phase2():  # scores operands (qT now fully loaded in phase 1)
        xfers = []
        for dc in range(0, DC, 2):
            xfers.append((kTb_s[:, dc : dc + 2, :], kTb_r[:, dc : dc + 2, :]))
        for dc in range(0, DC, 4):
            xfers.append((kT8_s[:, dc : dc + 4, :], kT8_r[:, dc : dc + 4, :]))
        emit_xfers(xfers, [nc.sync, nc.scalar, nc.gpsimd])

    def phase34():  # AX operand, then output projection operand
        xfers = []
        for c in range(0, KVC, 2):
            xfers.append((xv_s[:, c : c + 2, :], xv_r[:, c : c + 2, :]))
        xfers.append((bo2_s, bo2_bcast))
        for dc in range(0, DC, 2):
            xfers.append((n2_s[:, dc : dc + 2, :], n2_r[:, dc : dc + 2, :]))
        emit_xfers(xfers, [nc.sync, nc.gpsimd])

    # ---- Q'^T = M32.T @ qT ---------------------------------------------------
    qp = big.tile([P, DC, QL], CDT, tag="qp")  # Q'^T: [d'%128, d'//128, q]
    qp8 = None
    if KV8C < KVC:
        qp8 = big.tile([P, DC, QL], F8, tag="qp8", name="qp8")

    def qp_evac(ec, qb, ps):
        # one evac per psum bank, alternating engines, so the bank-release
        # drain is short; the fp8 copy for the DoubleRow scores is derived
        # from qp lazily (qp8_fill), off the bank-critical path.
        sl = slice(qb * N5, (qb + 1) * N5)
        if ec % 2 == 0:
            nc.scalar.activation(
                out=qp[:, ec, sl], in_=ps, func=AF.Identity, scale=1.0
            )
        else:
            nc.vector.tensor_copy(out=qp[:, ec, sl], in_=ps)

    def qp8_fill(qb):
        if qp8 is None:
            return
        sl = slice(qb * N5, (qb + 1) * N5)
        for ec in range(DC):
            nc.vector.tensor_copy(out=qp8[:, ec, sl], in_=qp[:, ec, sl])

    def qprime8(qb):
        pss = [
            qpool.tile([P, N5], F32, tag="qmm", name=f"qps{qb}_{ec}")
            for ec in range(DC)
        ]
        for dc in range(DC):
            for ec in range(DC):
                nc.tensor.matmul(
                    pss[ec],
                    lhsT=m32_s[:, dc, ec * P : (ec + 1) * P],
                    rhs=qT_s[:, dc, qb * N5 : (qb + 1) * N5],
                    start=(dc == 0),
                    stop=(dc == DC - 1),
                )
        for ec in range(DC):
            qp_evac(ec, qb, pss[ec])

    # ---- scores + exp + sums + AX, one kv pass per 512-query block ----------
    def scores_exp(qb):
        attnT = attn_pool.tile([P, KVC, N5], CDT, tag="attnT")
        qsl = slice(qb * N5, (qb + 1) * N5)
        for c in range(KVC):
            ps = psum.tile([P, N5], F32, tag="mm")
            if c < KV8C:  # bf16 path
                for dc in range(DC):
                    nc.tensor.matmul(
                        ps,
                        lhsT=kTb_s[:, dc, c * P : (c + 1) * P],
                        rhs=qp[:, dc, qsl],
                        start=(dc == 0),
                        stop=(dc == DC - 1),
                    )
            else:  # fp8 DoubleRow path: two d-chunks per instruction
                c8 = c - KV8C
                for dc in range(0, DC, 2):
                    nc.tensor.matmul(
                        ps,
                        lhsT=kT8_s[:, dc : dc + 2, c8 * P : (c8 + 1) * P],
                        rhs=qp8[:, dc : dc + 2, qsl],
                        start=(dc == 0),
                        stop=(dc == DC - 2),
                        perf_mode=DR,
                    )
            nc.scalar.activation(
                out=attnT[:, c, :],
                in_=ps,
                func=AF.Exp,
                scale=SCALE,
                bias=t2s_s[:, c : c + 1],
            )
        return attnT

    def sums_recip(attnT):
        ps_sum = psum_s.tile([P, N5 // P], F32, tag="sums")
        for s in range(N5 // P):
            for c in range(KVC):
                nc.tensor.matmul(
                    ps_sum[:, s : s + 1],
                    lhsT=attnT[:, c, s * P : (s + 1) * P],
                    rhs=ones[:, :1],
                    start=(c == 0),
                    stop=(c == KVC - 1),
                )
        r_s = evac.tile([P, N5 // P], F32, tag="recip")
        nc.vector.reciprocal(r_s, ps_sum)
        return r_s

    def ax_block(attnT):
        axT = attn_pool.tile([P, DC, N5], CDT, tag="axT")  # AX^T: [dv%128, m, q]
        for m in range(DC):
            ps = psum.tile([P, N5], F32, tag="mm")
            for c in range(KVC):
                nc.tensor.matmul(
                    ps,
                    lhsT=xv_s[:, c, m * P : (m + 1) * P],
                    rhs=attnT[:, c, :],
                    start=(c == 0),
                    stop=(c == KVC - 1),
                )
            nc.vector.tensor_copy(out=axT[:, m, :], in_=ps)
        return axT

    def out_block(qb, axT, r_s):
        for s in range(N5 // P):
            for nf in range(D // N5):
                ps = psum.tile([P, N5], F32, tag="mm")
                for m in range(DC):
                    nc.tensor.matmul(
                        ps,
                        lhsT=axT[:, m, s * P : (s + 1) * P],
                        rhs=n2_s[:, m, nf * N5 : (nf + 1) * N5],
                        start=(m == 0),
                        stop=(m == DC - 1),
                    )
                fin = evac.tile([P, N5], F32, tag="fin")
                nc.vector.scalar_tensor_tensor(
                    out=fin,
                    in0=ps,
                    scalar=r_s[:, s : s + 1],
                    in1=bo2_s[:, nf * N5 : (nf + 1) * N5],
                    op0=ALU.mult,
                    op1=ALU.add,
                )
                row0 = qb * N5 + s * P
                nc.sync.dma_start(
                    out=out[row0 : row0 + P, nf * N5 : (nf + 1) * N5], in_=fin
                )

    qprime8(0)
    phase2()
    qp8_fill(0)
    qprime8(1)
    phase34()
    qp8_fill(1)
    qpool_cm.__exit__(None, None, None)
    psum = ctx.enter_context(tc.tile_pool(name="psum", bufs=4, space="PSUM"))
    psum_s = ctx.enter_context(tc.tile_pool(name="psum_s", bufs=2, space="PSUM"))
    a0 = scores_exp(0)
    r0 = sums_recip(a0)
    x0 = ax_block(a0)
    out_block(0, x0, r0)
    a1 = scores_exp(1)
    r1 = sums_recip(a1)
    x1 = ax_block(a1)
    out_block(1, x1, r1)


def build_program():
    nc = bacc.Bacc(
        "TRN2", target_bir_lowering=False, debug=False, num_devices=NCORES
    )
    qT = nc.dram_tensor("qT", [D, QL], CDT, kind="ExternalInput").ap()
    kTb = nc.dram_tensor("kTb", [D, KVB], CDT, kind="ExternalInput").ap()
    kT8 = nc.dram_tensor("kT8", [D, SKV - KVB], F8, kind="ExternalInput").ap()
    xv = nc.dram_tensor("xv", [SKV, D], CDT, kind="ExternalInput").ap()
    m32 = nc.dram_tensor("m32", [D, D], CDT, kind="ExternalInput").ap()
    n2 = nc.dram_tensor("n2", [D, D], CDT, kind="ExternalInput").ap()
    t2s = nc.dram_tensor("t2s", [P, KVC], F32, kind="ExternalInput").ap()
    bo2 = nc.dram_tensor("bo2", [D], F32, kind="ExternalInput").ap()
    out = nc.dram_tensor("out", [QL, D], F32, kind="ExternalOutput").ap()

    with tile.TileContext(nc) as tc:
        with ExitStack() as ctx:
            _build_tile(ctx, tc, (qT, kTb, kT8, xv, m32, n2, t2s, bo2, out))
    nc.compile()
    return nc


def prep_in_maps(query, key, value, Wq, bq, Wk, bk, Wv, bv, Wo, bo):
    """Host-side shard prep: fold weights, slice, transpose, cast."""
    query = np.asarray(query, np.float32)
    key = np.asarray(key, np.float32)
    value = np.asarray(value, np.float32)
    Wq = np.asarray(Wq, np.float32)
    Wk = np.asarray(Wk, np.float32)
    Wv = np.asarray(Wv, np.float32)
    Wo = np.asarray(Wo, np.float32)
    bq = np.asarray(bq, np.float32)
    bv = np.asarray(bv, np.float32)
    bo = np.asarray(bo, np.float32)

    M32 = (Wq @ Wk.T) * MS
    N2 = Wv @ Wo
    ck = Wk @ bq  # per-kv score offset direction; zero when bq == 0
    shared = {
        "m32": M32.astype(NP_CDT),
        "n2": N2.astype(NP_CDT),
        "bo2": bv @ Wo + bo,
    }
    in_maps = []
    for b in range(B):
        kT = np.ascontiguousarray(key[b].T)
        kTbb = kT[:, :KVB].astype(NP_CDT)
        kT8b = kT[:, KVB:].astype(NP_F8)
        xvb = value[b].astype(NP_CDT)
        # pre-transposed to [P, KVC] so the DMA is 128 contiguous 64B runs
        # instead of 2048 four-byte runs (descriptor-generation cost)
        t2sb = np.ascontiguousarray(
            (SCALE * (key[b] @ ck)).astype(np.float32).reshape(KVC, P).T
        )
        for h in range(2):
            qTb = np.ascontiguousarray(query[b, h * QL : (h + 1) * QL].T).astype(
                NP_CDT
            )
            in_maps.append(
                {"qT": qTb, "kTb": kTbb, "kT8": kT8b, "xv": xvb, "t2s": t2sb,
                 **shared}
            )
    return in_maps


_NC_CACHE = None


def _get_nc():
    global _NC_CACHE
    if _NC_CACHE is None:
        _NC_CACHE = build_program()
    return _NC_CACHE


def run(inputs, **run_kwargs):
    nc = _get_nc()
    in_maps = prep_in_maps(**inputs)
    res = run_bass_kernel_spmd(nc, in_maps, core_ids=list(range(NCORES)), **run_kwargs)
    out = np.empty((B, SQ, D), np.float32)
    for b in range(B):
        for h in range(2):
            out[b, h * QL : (h + 1) * QL] = res.results[2 * b + h]["out"]
    return out, res


def kernel(query, key, value, Wq, bq, Wk, bk, Wv, bv, Wo, bo):
    out, _ = run(
        dict(
            query=query, key=key, value=value, Wq=Wq, bq=bq, Wk=Wk, bk=bk,
            Wv=Wv, bv=bv, Wo=Wo, bo=bo,
        )
    )
    return out


if __name__ == "__main__":
    rng = np.random.default_rng(0)
    ins = {
        "query": rng.standard_normal((B, SQ, D), dtype=np.float32),
        "key": rng.standard_normal((B, SKV, D), dtype=np.float32),
        "value": rng.standard_normal((B, SKV, D), dtype=np.float32),
        "Wq": (rng.standard_normal((D, D), dtype=np.float32) * 0.02),
        "bq": np.zeros(D, np.float32),
        "Wk": (rng.standard_normal((D, D), dtype=np.float32) * 0.02),
        "bk": np.zeros(D, np.float32),
        "Wv": (rng.standard_normal((D, D), dtype=np.float32) * 0.02),
        "bv": np.zeros(D, np.float32),
        "Wo": (rng.standard_normal((D, D), dtype=np.float32) * 0.02),
        "bo": np.zeros(D, np.float32),
    }
    out = kernel(**ins)
    print("kernel ran, out shape", out.shape)
